# revision 1
# baseline (speedup 1.0000x reference)
import sys

for _p in ("/opt/trn_rl_repo", "/opt/pypackages"):
    if _p not in sys.path:
        sys.path.append(_p)

import numpy as np
import concourse.bass as bass
import concourse.tile as tile
from concourse import mybir
from concourse.bass_utils import run_bass_kernel_spmd

AF = mybir.ActivationFunctionType
ALU = mybir.AluOpType
F32R = mybir.dt.float32r
F32 = mybir.dt.float32
I32 = mybir.dt.int32

B, L, C, D, H, DEPTH, FF, TE = 8, 5160, 2, 256, 8, 8, 1024, 256
HD = D // H
NCORES = 8
EPS = 1e-5
PI = float(np.pi)
ISQ = float(1.0 / np.sqrt(HD))

CHUNKS = [(i * 512, 512) for i in range(10)] + [(5120, 40)]
LCH = [(i * 128, 128) for i in range(40)] + [(5120, 40)]
NL = len(LCH)

TRACE = False
LAST = None


def split_excess_waits(nc, limit=1):
    fn = nc.m.functions[0]
    blocks = getattr(fn, "instruction_blocks", None) or getattr(fn, "blocks")
    for bb in blocks:
        insts = bb.instructions
        out = []
        for inst in insts:
            si = inst.sync_info
            waits = list(si.on_wait) if si is not None and si.on_wait else []
            if len(waits) > limit:
                keep = waits[-limit:]
                excess = waits[:-limit]
                for i in range(0, len(excess), limit):
                    nop = mybir.InstNoOp(
                        name=nc.get_next_instruction_name(),
                        sync_info=mybir.SyncInfo(
                            on_wait=excess[i:i + limit], on_update=[]
                        ),
                        bass_nofuse=True,
                        engine=inst.engine,
                    )
                    nc.register_instruction(nop)
                    out.append(nop)
                si.on_wait = keep
            out.append(inst)
        if len(out) != len(insts):
            insts[:] = out
    return nc


def build_nc(depth=DEPTH):
    nc = bass.Bass(target_bir_lowering=False, trn_type="TRN2")
    V = nc.vector
    S = nc.scalar
    G = nc.gpsimd
    T = nc.tensor

    def mmv(out, lhsT, rhs2, start, stop):
        T.matmul(out, lhsT, rhs2, start=start, stop=stop)

    def dup2(pool, src1, name):
        v2 = pool.tile([src1.shape[0], 2], F32R, tag=name, name=name)
        V.tensor_copy(v2[:], src1.to_broadcast([src1.shape[0], 2]))
        return v2

    d_xT = nc.dram_tensor("xT", [C, L], F32R, kind="ExternalInput")
    d_t = nc.dram_tensor("tval", [1, 1], I32, kind="ExternalInput")
    d_posT = nc.dram_tensor("posT", [D, L], F32, kind="ExternalInput")
    d_inwT = nc.dram_tensor("inwT", [C, D], F32R, kind="ExternalInput")
    d_inb = nc.dram_tensor("inb", [D, 1], F32, kind="ExternalInput")
    d_freqs = nc.dram_tensor("freqs", [TE // 2, 2], F32, kind="ExternalInput")
    d_tp1T = nc.dram_tensor("tp1T", [TE, D], F32R, kind="ExternalInput")
    d_tp1b = nc.dram_tensor("tp1b", [D, 1], F32, kind="ExternalInput")
    d_tp2T = nc.dram_tensor("tp2T", [D, D], F32R, kind="ExternalInput")
    d_tp2b = nc.dram_tensor("tp2b", [D, 1], F32, kind="ExternalInput")
    d_cls = nc.dram_tensor("clsv", [D, 1], F32R, kind="ExternalInput")
    d_qkvoT = nc.dram_tensor("qkvoT", [DEPTH, 4, D, D], F32R, kind="ExternalInput")
    d_attnb = nc.dram_tensor("attnb", [DEPTH, 4, D, 1], F32, kind="ExternalInput")
    d_modT = nc.dram_tensor("modT", [DEPTH, 3, D, 3 * D], F32R, kind="ExternalInput")
    d_modb = nc.dram_tensor("modb", [DEPTH, 3, 3 * D, 1], F32, kind="ExternalInput")
    d_lng = nc.dram_tensor("lng", [DEPTH, 3, D, 1], F32, kind="ExternalInput")
    d_lnb = nc.dram_tensor("lnb", [DEPTH, 3, D, 1], F32, kind="ExternalInput")
    d_w1T = nc.dram_tensor("w1T", [DEPTH, 2, D, FF], F32R, kind="ExternalInput")
    d_b1 = nc.dram_tensor("b1", [DEPTH, 2, FF, 1], F32, kind="ExternalInput")
    d_w2T = nc.dram_tensor("w2T", [DEPTH, 2, FF, D], F32R, kind="ExternalInput")
    d_b2 = nc.dram_tensor("b2", [DEPTH, 2, D, 1], F32, kind="ExternalInput")
    d_fing = nc.dram_tensor("fing", [D, 1], F32, kind="ExternalInput")
    d_finb = nc.dram_tensor("finb", [D, 1], F32, kind="ExternalInput")
    d_outwT = nc.dram_tensor("outwT", [D, C], F32R, kind="ExternalInput")
    d_outb = nc.dram_tensor("outb", [C, 1], F32, kind="ExternalInput")
    d_ident = nc.dram_tensor("ident", [8, 8], F32R, kind="ExternalInput")
    d_ident128 = nc.dram_tensor("ident128", [128, 128], F32R, kind="ExternalInput")
    d_sel = nc.dram_tensor("selw", [2, 256], F32R, kind="ExternalInput")
    d_ones = nc.dram_tensor("onesw", [128, 512], F32R, kind="ExternalInput")
    d_zeros = nc.dram_tensor("zerosw", [128, NL * 8], F32R, kind="ExternalInput")
    d_onessc = nc.dram_tensor("onessc", [128, 1], F32R, kind="ExternalInput")
    d_outT = nc.dram_tensor("outT", [C, L], F32, kind="ExternalOutput")

    def col2(dram_ap, groups):
        return dram_ap[:, 0].rearrange("(g p) -> p g", p=128)

    def ld_split(dst, dram2d, g):
        x = dram2d.shape[1]
        nc.sync.dma_start(
            dst.rearrange("p (g x) -> p g x", g=g),
            dram2d.rearrange("(g p) x -> p g x", p=128))

    with tile.TileContext(nc) as tc:
        with tc.tile_pool(name="state", bufs=1) as st, \
             tc.tile_pool(name="wts", bufs=2) as wp, \
             tc.tile_pool(name="vecs", bufs=2) as vp, \
             tc.tile_pool(name="chk", bufs=2) as cp, \
             tc.tile_pool(name="ph1", bufs=2, space="PSUM") as ph1, \
             tc.tile_pool(name="ph2", bufs=2, space="PSUM") as ph2, \
             tc.tile_pool(name="py", bufs=1, space="PSUM") as pyp, \
             tc.tile_pool(name="pmisc", bufs=3, space="PSUM") as pm:

            tokT = [st.tile([128, L], F32R, tag=f"tok{d}", name=f"tok{d}") for d in range(2)]
            pT = st.tile([128, NL * 8], F32R, tag="pT", name="pT")
            muS = st.tile([128, 48], F32, tag="muS", name="muS")
            m2S = st.tile([128, 48], F32, tag="m2S", name="m2S")
            stat2 = st.tile([128, 2 * NL], F32R, tag="stat2", name="stat2")
            murT = st.tile([2, NL * 128], F32R, tag="murT", name="murT")
            sel = st.tile([2, 256], F32R, tag="sel", name="sel")
            onesr = st.tile([1, 512], F32R, tag="onesr", name="onesr")
            onescol = st.tile([128, 2], F32R, tag="onescol", name="onescol")
            cls = [st.tile([128, 1], F32R, tag=f"cls{d}", name=f"cls{d}") for d in range(2)]
            temb = [st.tile([128, 2], F32R, tag=f"temb{d}", name=f"temb{d}") for d in range(2)]
            stm = [st.tile([128, 2], F32R, tag=f"stm{d}", name=f"stm{d}") for d in range(2)]
            modpre = st.tile([128, DEPTH * 12], F32, tag="modpre", name="modpre")
            abpre = st.tile([128, DEPTH * 8], F32, tag="abpre", name="abpre")
            identsb = st.tile([8, 8], F32R, tag="ident", name="ident")
            id128 = st.tile([128, 128], F32R, tag="id128", name="id128")
            onessc = st.tile([128, 1], F32R, tag="onessc", name="onessc")
            lngsb = st.tile([128, DEPTH * 6], F32, tag="lngsb", name="lngsb")
            lnbsb = st.tile([128, DEPTH * 6], F32, tag="lnbsb", name="lnbsb")
            NTOKL = 4
            tokL = [st.tile([128, 264], F32R, tag=f"tokL{j}", name=f"tokL{j}")
                    for j in range(NTOKL)]

            nc.sync.dma_start(identsb[:], d_ident[:, :])
            nc.sync.dma_start(id128[:], d_ident128[:, :])
            nc.sync.dma_start(onessc[:], d_onessc[:, :])
            nc.sync.dma_start(onesr[:], d_ones[0:1, :])
            nc.sync.dma_start(onescol[:], d_ones[:, 0:2])
            nc.sync.dma_start(pT[:], d_zeros[:, :])
            nc.sync.dma_start(sel[:], d_sel[:, :])
            for j in range(NTOKL):
                nc.sync.dma_start(tokL[j][:, 256:258], d_ones[:, 0:2])
            for dt in range(2):
                nc.sync.dma_start(
                    lngsb[:].rearrange("p (i g dt) -> p i g dt",
                                       i=DEPTH, g=3)[:, :, :, dt],
                    d_lng[:, :, 128 * dt:128 * dt + 128, 0].rearrange(
                        "i g p -> p i g"))
                nc.sync.dma_start(
                    lnbsb[:].rearrange("p (i g dt) -> p i g dt",
                                       i=DEPTH, g=3)[:, :, :, dt],
                    d_lnb[:, :, 128 * dt:128 * dt + 128, 0].rearrange(
                        "i g p -> p i g"))
            nc.sync.dma_start(cls[0][:], d_cls[0:128, :])
            nc.sync.dma_start(cls[1][:], d_cls[128:256, :])

            with tc.tile_pool(name="pre", bufs=1) as pre:
                tfl = pre.tile([1, 2], F32R, tag="tfl", name="tfl")
                traw = pre.tile([1, 1], I32, tag="traw", name="traw")
                nc.sync.dma_start(traw[:], d_t[:, :])
                V.tensor_copy(tfl[:], traw[:].to_broadcast([1, 2]))
                tb = pm.tile([128, 2], F32, tag="ps", name="tb")
                mmv(tb[:], onesr[0:1, 0:128], tfl[:], start=True, stop=True)
                fsb = pre.tile([128, 2], F32, tag="fsb", name="fsb")
                nc.sync.dma_start(fsb[:], d_freqs[:, :])
                ang = pre.tile([128, 1], F32, tag="ang", name="ang")
                ang2 = pre.tile([128, 1], F32, tag="ang2", name="ang2")
                V.tensor_tensor(ang[:], tb[:, 0:1], fsb[:, 0:1], ALU.mult)
                V.tensor_tensor(ang2[:], tb[:, 0:1], fsb[:, 1:2], ALU.mult)
                V.tensor_tensor(ang[:], ang[:], ang2[:], ALU.add)
                dsc = pre.tile([128, 1], F32, tag="dsc", name="dsc")
                qi = pre.tile([128, 1], I32, tag="qi", name="qi")
                qf = pre.tile([128, 1], F32, tag="qf", name="qf")
                msk = pre.tile([128, 1], F32, tag="msk", name="msk")
                TWO_PI = 2 * PI

                def mod2pi(dst, shift):
                    V.tensor_scalar(dst[:], ang[:], shift, None, ALU.add)
                    V.tensor_scalar(dsc[:], dst[:], 1.0 / TWO_PI, 0.5,
                                    ALU.mult, ALU.subtract)
                    V.tensor_copy(qi[:], dsc[:])
                    V.tensor_copy(qf[:], qi[:])
                    V.scalar_tensor_tensor(dst[:], qf[:], -TWO_PI, dst[:],
                                           ALU.mult, ALU.add)
                    V.tensor_scalar(msk[:], dst[:], TWO_PI, None, ALU.is_ge)
                    V.scalar_tensor_tensor(dst[:], msk[:], -TWO_PI, dst[:],
                                           ALU.mult, ALU.add)
                    V.tensor_scalar(msk[:], dst[:], 0.0, None, ALU.is_lt)
                    V.scalar_tensor_tensor(dst[:], msk[:], TWO_PI, dst[:],
                                           ALU.mult, ALU.add)
                    V.tensor_scalar(dst[:], dst[:], PI, None, ALU.subtract)

                m1 = pre.tile([128, 1], F32, tag="m1", name="m1")
                mod2pi(m1, PI)
                m2 = pre.tile([128, 1], F32, tag="m2", name="m2")
                mod2pi(m2, 1.5 * PI)
                sinf = pre.tile([128, 2], F32R, tag="sinf", name="sinf")
                cosf = pre.tile([128, 2], F32R, tag="cosf", name="cosf")
                S.activation(sinf[:], m1[:].to_broadcast([128, 2]), AF.Sin)
                S.activation(cosf[:], m2[:].to_broadcast([128, 2]), AF.Sin)

                ttp1 = pre.tile([128, 512], F32R, tag="ttp1", name="ttp1")
                ld_split(ttp1[:], d_tp1T, 2)
                ttp2 = pre.tile([128, 512], F32R, tag="ttp2", name="ttp2")
                ld_split(ttp2[:], d_tp2T, 2)
                tp1b = pre.tile([128, 2], F32, tag="tp1b", name="tp1b")
                nc.sync.dma_start(tp1b[:], col2(d_tp1b, 2))
                tp2b = pre.tile([128, 2], F32, tag="tp2b", name="tp2b")
                nc.sync.dma_start(tp2b[:], col2(d_tp2b, 2))

                st1 = [pre.tile([128, 2], F32R, tag=f"st1{m}", name=f"st1{m}") for m in range(2)]
                for m in range(2):
                    ps = pm.tile([128, 2], F32, tag="ps", name="ps")
                    mmv(ps[:], ttp1[:, 128 * m:128 * m + 128], sinf[:],
                        start=True, stop=False)
                    mmv(ps[:], ttp1[:, 256 + 128 * m:256 + 128 * m + 128],
                        cosf[:], start=False, stop=True)
                    S.activation(st1[m][:], ps[:], AF.Silu, bias=tp1b[:, m:m + 1])
                for m in range(2):
                    ps = pm.tile([128, 2], F32, tag="ps", name="ps")
                    mmv(ps[:], ttp2[:, 128 * m:128 * m + 128], st1[0][:],
                        start=True, stop=False)
                    mmv(ps[:], ttp2[:, 256 + 128 * m:256 + 128 * m + 128],
                        st1[1][:], start=False, stop=True)
                    S.activation(temb[m][:], ps[:], AF.Identity,
                                 bias=tp2b[:, m:m + 1])
                    S.activation(stm[m][:], temb[m][:], AF.Silu)

                for i in range(depth):
                    for g in range(2):
                        tmg = pre.tile([128, 1536], F32R, tag="tmg", name="tmg")
                        ld_split(tmg[:], d_modT[i, g], 2)
                        tmb = pre.tile([128, 6], F32, tag="tmb", name="tmb")
                        nc.sync.dma_start(tmb[:], col2(d_modb[i, g], 6))
                        for m in range(6):
                            ps = pm.tile([128, 2], F32, tag="ps", name="ps")
                            mmv(ps[:], tmg[:, 128 * m:128 * m + 128],
                                stm[0][:], start=True, stop=False)
                            mmv(ps[:], tmg[:, 768 + 128 * m:768 + 128 * m + 128],
                                stm[1][:], start=False, stop=True)
                            colm = i * 12 + g * 6 + m
                            V.tensor_scalar(modpre[:, colm:colm + 1], ps[:, 0:1],
                                            tmb[:, m:m + 1], None, ALU.add)
                        scr = pre.tile([128, 1], F32, tag="scr", name="scr")
                        for dt in range(2):
                            scol = i * 12 + g * 6 + dt
                            shcol = i * 12 + g * 6 + 2 + dt
                            lcol = i * 6 + g * 2 + dt
                            acol = i * 8 + g * 4 + dt
                            bcol = i * 8 + g * 4 + 2 + dt
                            V.tensor_scalar(scr[:], modpre[:, scol:scol + 1],
                                            1.0, None, ALU.add)
                            V.tensor_tensor(abpre[:, acol:acol + 1], scr[:],
                                            lngsb[:, lcol:lcol + 1], ALU.mult)
                            V.tensor_tensor(abpre[:, bcol:bcol + 1], scr[:],
                                            lnbsb[:, lcol:lcol + 1], ALU.mult)
                            V.tensor_tensor(abpre[:, bcol:bcol + 1],
                                            abpre[:, bcol:bcol + 1],
                                            modpre[:, shcol:shcol + 1], ALU.add)

            with tc.tile_pool(name="pre2", bufs=1) as pre:
                inwsb = pre.tile([C, D], F32R, tag="inwsb", name="inwsb")
                nc.sync.dma_start(inwsb[:], d_inwT[:, :])
                inbsb = pre.tile([128, 2], F32, tag="inbsb", name="inbsb")
                nc.sync.dma_start(inbsb[:], col2(d_inb, 2))
                for (o, w) in CHUNKS:
                    xtc = pre.tile([C, 512], F32R, tag="xtc", name="xtc")
                    nc.sync.dma_start(xtc[:, 0:w], d_xT[:, o:o + w])
                    for dt in range(2):
                        ppc = pre.tile([128, 512], F32, tag=f"ppc{dt}", name=f"ppc{dt}")
                        nc.sync.dma_start(ppc[:, 0:w],
                                          d_posT[128 * dt:128 * dt + 128, o:o + w])
                        ps = pm.tile([128, 512], F32, tag="ps", name="ps")
                        T.matmul(ps[:, 0:w], inwsb[:, 128 * dt:128 * dt + 128],
                                 xtc[:, 0:w], start=True, stop=True)
                        V.scalar_tensor_tensor(
                            tokT[dt][:, o:o + w], ps[:, 0:w],
                            inbsb[:, dt:dt + 1], ppc[:, 0:w], ALU.add, ALU.add)

            def cls_ln(i, g, out_tag):
                csc = [vp.tile([128, 2], F32R, tag=f"csc{d}", name=f"csc{d}") for d in range(2)]
                for d in range(2):
                    V.tensor_copy(csc[d][:, 0:1], onessc[:, 0:1])
                    V.tensor_scalar(csc[d][:, 1:2], cls[d][:], 1.0 / 256, None,
                                    ALU.mult)
                ps = pm.tile([1, 2], F32, tag="ps", name="ps")
                for d in range(2):
                    mmv(ps[0:1, 0:2], cls[d][:], csc[d][:, 0:2],
                        start=(d == 0), stop=(d == 1))
                mc = vp.tile([1, 8], F32R, tag="mc", name="mc")
                V.tensor_copy(mc[0:1, 0:2], ps[0:1, 0:2])
                V.tensor_tensor(mc[0:1, 2:3], mc[0:1, 0:1], mc[0:1, 0:1],
                                ALU.mult)
                V.scalar_tensor_tensor(mc[0:1, 3:4], mc[0:1, 1:2], EPS,
                                       mc[0:1, 2:3], ALU.add, ALU.subtract)
                with nc.allow_low_precision(reason="fp22 cls LN stats"):
                    V.reciprocal(mc[0:1, 3:4], mc[0:1, 3:4])
                S.activation(mc[0:1, 3:4], mc[0:1, 3:4], AF.Sqrt)
                V.tensor_copy(mc[0:1, 4:6], mc[0:1, 0:1].to_broadcast([1, 2]))
                V.tensor_copy(mc[0:1, 6:8], mc[0:1, 3:4].to_broadcast([1, 2]))
                mcb = pm.tile([128, 2], F32, tag="ps", name="mcb")
                rcb = pm.tile([128, 2], F32, tag="ps", name="rcb")
                mmv(mcb[:], onesr[0:1, 0:128], mc[0:1, 4:6],
                    start=True, stop=True)
                mmv(rcb[:], onesr[0:1, 0:128], mc[0:1, 6:8],
                    start=True, stop=True)
                hc = [vp.tile([128, 2], F32R, tag=f"{out_tag}{d}", name=f"{out_tag}{d}") for d in range(2)]
                for d in range(2):
                    acol = i * 8 + g * 4 + d
                    bcol = i * 8 + g * 4 + 2 + d
                    V.tensor_tensor(hc[d][:], cls[d][:].to_broadcast([128, 2]),
                                    mcb[:], ALU.subtract)
                    V.tensor_tensor(hc[d][:], hc[d][:], rcb[:], ALU.mult)
                    V.scalar_tensor_tensor(
                        hc[d][:], hc[d][:], abpre[:, acol:acol + 1],
                        abpre[:, bcol:bcol + 1].to_broadcast([128, 2]),
                        ALU.mult, ALU.add)
                return hc

            for i in range(depth):
                qkvo = wp.tile([128, 2048], F32R, tag="qkvo", name="qkvo")
                for dt in range(2):
                    nc.sync.dma_start(
                        qkvo[:, 1024 * dt:1024 * dt + 1024].rearrange(
                            "p (w x) -> p w x", w=4),
                        d_qkvoT[i][:, 128 * dt:128 * dt + 128, :].rearrange(
                            "w p x -> p w x"))
                w1 = wp.tile([128, 2048], F32R, tag="w1", name="w1")
                ld_split(w1[:], d_w1T[i, 1], 2)
                w2 = wp.tile([128, 2048], F32R, tag="w2", name="w2")
                ld_split(w2[:], d_w2T[i, 1], 8)
                w1c = wp.tile([128, 2048], F32R, tag="w1c", name="w1c", bufs=1)
                ld_split(w1c[:], d_w1T[i, 0], 2)
                w2c = wp.tile([128, 2048], F32R, tag="w2c", name="w2c", bufs=1)
                ld_split(w2c[:], d_w2T[i, 0], 8)
                mod2 = wp.tile([128, 1536], F32R, tag="mod2", name="mod2", bufs=1)
                ld_split(mod2[:], d_modT[i, 2], 2)
                tattnb = vp.tile([128, 8], F32, tag="tattnb", name="tattnb")
                nc.sync.dma_start(
                    tattnb[:].rearrange("p (w dt) -> p w dt", w=4),
                    d_attnb[i][:, :, 0].rearrange("w (dt p) -> p w dt", p=128))
                tb1 = vp.tile([128, 8], F32, tag="tb1", name="tb1")
                nc.sync.dma_start(tb1[:], col2(d_b1[i, 1], 8))
                tb1c = vp.tile([128, 8], F32, tag="tb1c", name="tb1c")
                nc.sync.dma_start(tb1c[:], col2(d_b1[i, 0], 8))
                b2row = vp.tile([1, 256], F32R, tag="b2row", name="b2row", bufs=1)
                nc.sync.dma_start(b2row[:], d_b2[i, 1].rearrange("d o -> o d").bitcast(F32R))
                tb2c = vp.tile([128, 2], F32, tag="tb2c", name="tb2c")
                nc.sync.dma_start(tb2c[:], col2(d_b2[i, 0], 2))
                tmodb2 = vp.tile([128, 6], F32, tag="tmodb2", name="tmodb2")
                nc.sync.dma_start(tmodb2[:], col2(d_modb[i, 2], 6))

                hc = cls_ln(i, 0, "hca")
                Qm = [vp.tile([128, 8], F32R, tag=f"qm{d}", name=f"qm{d}") for d in range(2)]
                for d in range(2):
                    qp = pm.tile([128, 2], F32, tag="ps", name="ps")
                    mmv(qp[:], qkvo[:, 128 * d:128 * d + 128],
                        hc[0][:], start=True, stop=False)
                    mmv(qp[:], qkvo[:, 1024 + 128 * d:1024 + 128 * d + 128],
                        hc[1][:], start=False, stop=True)
                    nc.sync.dma_start(Qm[d][:], d_zeros[:, 0:8])
                    for hh in range(4):
                        r0 = 32 * hh
                        col = 4 * d + hh
                        V.tensor_scalar(Qm[d][r0:r0 + 32, col:col + 1],
                                        qp[r0:r0 + 32, 0:1],
                                        tattnb[r0:r0 + 32, 0 + d:d + 1],
                                        None, ALU.add)
                wq = [vp.tile([128, 8], F32R, tag=f"wq{d}", name=f"wq{d}") for d in range(2)]
                for cb in range(2):
                    wqp = pm.tile([128, 8], F32, tag="ps", name="ps")
                    for fb in range(2):
                        T.matmul(wqp[:],
                                 qkvo[:, 1024 * fb + 256 + 128 * cb:
                                      1024 * fb + 256 + 128 * cb + 128],
                                 Qm[fb][:], start=(fb == 0), stop=(fb == 1))
                    V.tensor_copy(wq[cb][:], wqp[:])

                y_ps = pyp.tile([8, 258], F32, tag="y", name="y_ps")
                for g in range(11):
                    ns = min(4, NL - 4 * g)
                    sT = pm.tile([128, 32], F32, tag="ps", name="sT")
                    for s in range(ns):
                        ci = 4 * g + s
                        o, w = LCH[ci]
                        slot = tokL[ci % NTOKL]
                        tpp = pm.tile([128, 256], F32, tag="ps", name="tpp")
                        for dt in range(2):
                            T.transpose(tpp[0:w, 128 * dt:128 * dt + 128],
                                        tokT[dt][:, o:o + w].bitcast(F32),
                                        id128[:, :].bitcast(F32))
                        S.copy(slot[0:w, 0:256], tpp[0:w, 0:256])
                        for cb in range(2):
                            T.matmul(sT[0:w, 8 * s:8 * s + 8],
                                     tokT[cb][:, o:o + w], wq[cb][:],
                                     start=(cb == 0), stop=(cb == 1))
                        V.tensor_reduce(muS[:, ci:ci + 1], slot[:, 0:256],
                                        mybir.AxisListType.X, ALU.add)
                        scrq = cp.tile([128, 256], F32R, tag="scr2", name="scrq")
                        S.activation(scrq[:], slot[:, 0:256], AF.Square,
                                     accum_out=m2S[:, ci:ci + 1])
                    wg = 128 if ns == 4 else LCH[4 * g][1]
                    S.activation(pT[0:wg, 32 * g:32 * g + 8 * ns],
                                 sT[0:wg, 0:8 * ns], AF.Exp, scale=ISQ)
                    for s in range(ns):
                        ci = 4 * g + s
                        T.matmul(y_ps[:, 0:258], pT[:, 8 * ci:8 * ci + 8],
                                 tokL[ci % NTOKL][:, 0:258],
                                 start=(ci == 0), stop=(ci == NL - 1))

                V.tensor_scalar(muS[:, 0:NL], muS[:, 0:NL], 1.0 / 256,
                                None, ALU.mult)
                V.tensor_scalar(m2S[:, 0:NL], m2S[:, 0:NL], 1.0 / 256,
                                None, ALU.mult)
                musq = vp.tile([128, 48], F32, tag="musq", name="musq")
                V.tensor_tensor(musq[:, 0:NL], muS[:, 0:NL], muS[:, 0:NL],
                                ALU.mult)
                var48 = vp.tile([128, 48], F32, tag="var48", name="var48")
                V.scalar_tensor_tensor(var48[:, 0:NL], m2S[:, 0:NL], EPS,
                                       musq[:, 0:NL], ALU.add, ALU.subtract)
                V.reciprocal(var48[:, 0:NL], var48[:, 0:NL])
                st2v = stat2[:].rearrange("p (ci two) -> p two ci", two=2)
                V.tensor_copy(st2v[:, 0, :], muS[:, 0:NL])
                S.activation(st2v[:, 1, :], var48[:, 0:NL], AF.Sqrt)
                for g in range(11):
                    ns = min(4, NL - 4 * g)
                    mt = pm.tile([2, 512], F32, tag="ps", name="mt")
                    for s in range(ns):
                        ci = 4 * g + s
                        T.transpose(mt[0:2, 128 * s:128 * s + 128],
                                    stat2[:, 2 * ci:2 * ci + 2].bitcast(F32),
                                    id128[:, :].bitcast(F32))
                    V.tensor_copy(murT[0:2, 512 * g:512 * g + 128 * ns],
                                  mt[0:2, 0:128 * ns])

                srec = vp.tile([8, 1], F32, tag="srec", name="srec")
                V.reciprocal(srec[:], y_ps[:, 256:257])
                ysc = vp.tile([8, 256], F32R, tag="ysc", name="ysc")
                V.tensor_scalar(ysc[:], y_ps[:, 0:256], srec[:], None, ALU.mult)
                yT = [vp.tile([128, 8], F32R, tag=f"yT{d}", name=f"yT{d}") for d in range(2)]
                for cb in range(2):
                    ytp = pm.tile([128, 8], F32R, tag="ps", name="ytp")
                    T.transpose(ytp[0:128, 0:8], ysc[:, 128 * cb:128 * cb + 128],
                                identsb[:, :])
                    V.tensor_copy(yT[cb][:], ytp[:])
                OF = pyp.tile([8, 256], F32, tag="y", name="OF")
                for cb in range(2):
                    T.matmul(OF[:, :], yT[cb][:],
                             qkvo[:, 1024 * cb + 512:1024 * cb + 768],
                             start=(cb == 0), stop=(cb == 1))
                OFs = vp.tile([8, 256], F32R, tag="OFs", name="OFs")
                V.tensor_copy(OFs[:], OF[:, :])

                afl = [vp.tile([128, 2], F32R, tag=f"afl{d}", name=f"afl{d}") for d in range(2)]
                for d in range(2):
                    tpa = pm.tile([128, 8], F32R, tag="ps", name="tpa")
                    T.transpose(tpa[0:128, 0:8], OFs[:, 128 * d:128 * d + 128],
                                identsb[0:8, 0:8])
                    for hh in range(4):
                        r0 = 32 * hh
                        col = 4 * d + hh
                        V.tensor_scalar(
                            afl[d][r0:r0 + 32, 0:2],
                            tpa[r0:r0 + 32, col:col + 1].to_broadcast([32, 2]),
                            tattnb[r0:r0 + 32, 4 + d:4 + d + 1],
                            None, ALU.add)
                for d in range(2):
                    op_ = pm.tile([128, 2], F32, tag="ps", name="ps")
                    mmv(op_[:], qkvo[:, 768 + 128 * d:768 + 128 * d + 128],
                        afl[0][:], start=True, stop=False)
                    mmv(op_[:], qkvo[:, 1024 + 768 + 128 * d:1024 + 768 + 128 * d + 128],
                        afl[1][:], start=False, stop=True)
                    gcol = i * 12 + 0 * 6 + 4 + d
                    V.scalar_tensor_tensor(cls[d][:], op_[:, 0:1],
                                           modpre[:, gcol:gcol + 1],
                                           cls[d][:], ALU.mult, ALU.add)
                    bog = vp.tile([128, 1], F32, tag="bog", name="bog")
                    V.tensor_tensor(bog[:], tattnb[:, 6 + d:6 + d + 1],
                                    modpre[:, gcol:gcol + 1], ALU.mult)
                    V.tensor_tensor(cls[d][:], cls[d][:], bog[:], ALU.add)

                hc2 = cls_ln(i, 1, "hcm")
                ac = [vp.tile([128, 2], F32R, tag=f"ac{m}", name=f"ac{m}") for m in range(8)]
                for m in range(8):
                    ps = pm.tile([128, 2], F32, tag="ps", name="ps")
                    mmv(ps[:], w1c[:, 128 * m:128 * m + 128], hc2[0][:],
                        start=True, stop=False)
                    mmv(ps[:], w1c[:, 1024 + 128 * m:1024 + 128 * m + 128],
                        hc2[1][:], start=False, stop=True)
                    S.activation(ac[m][:], ps[:], AF.Gelu, bias=tb1c[:, m:m + 1])
                for d in range(2):
                    ps = pm.tile([128, 2], F32, tag="ps", name="ps")
                    for k in range(8):
                        mmv(ps[:], w2c[:, 256 * k + 128 * d:256 * k + 128 * d + 128],
                            ac[k][:], start=(k == 0), stop=(k == 7))
                    gcol = i * 12 + 1 * 6 + 4 + d
                    V.scalar_tensor_tensor(cls[d][:], ps[:, 0:1],
                                           modpre[:, gcol:gcol + 1],
                                           cls[d][:], ALU.mult, ALU.add)
                    bog = vp.tile([128, 1], F32, tag="bog", name="bog")
                    V.tensor_tensor(bog[:], tb2c[:, d:d + 1],
                                    modpre[:, gcol:gcol + 1], ALU.mult)
                    V.tensor_tensor(cls[d][:], cls[d][:], bog[:], ALU.add)

                sc2 = [vp.tile([128, 2], F32R, tag=f"sc2{d}", name=f"sc2{d}") for d in range(2)]
                for d in range(2):
                    cond = vp.tile([128, 2], F32R, tag=f"cond{d}", name=f"cond{d}")
                    V.tensor_tensor(cond[:], temb[d][:],
                                    cls[d][:].to_broadcast([128, 2]), ALU.add)
                    S.activation(sc2[d][:], cond[:], AF.Silu)
                mvec = vp.tile([128, 6], F32, tag="mvec", name="mvec")
                for m in range(6):
                    ps = pm.tile([128, 2], F32, tag="ps", name="ps")
                    mmv(ps[:], mod2[:, 128 * m:128 * m + 128], sc2[0][:],
                        start=True, stop=False)
                    mmv(ps[:], mod2[:, 768 + 128 * m:768 + 128 * m + 128],
                        sc2[1][:], start=False, stop=True)
                    V.tensor_scalar(mvec[:, m:m + 1], ps[:, 0:1], tmodb2[:, m:m + 1],
                                    None, ALU.add)
                av = vp.tile([128, 2], F32, tag="av", name="av")
                bv = vp.tile([128, 4], F32R, tag="bv", name="bv")
                scr2 = vp.tile([128, 1], F32, tag="scr2", name="scr2")
                for d in range(2):
                    lcol = i * 6 + 2 * 2 + d
                    V.tensor_scalar(scr2[:], mvec[:, d:d + 1], 1.0, None, ALU.add)
                    V.tensor_tensor(av[:, d:d + 1], scr2[:],
                                    lngsb[:, lcol:lcol + 1], ALU.mult)
                    V.tensor_tensor(bv[:, 2 * d:2 * d + 2],
                                    scr2[:].to_broadcast([128, 2]),
                                    lnbsb[:, lcol:lcol + 1].to_broadcast([128, 2]),
                                    ALU.mult)
                    V.tensor_tensor(bv[:, 2 * d:2 * d + 2], bv[:, 2 * d:2 * d + 2],
                                    mvec[:, 2 + d:3 + d].to_broadcast([128, 2]),
                                    ALU.add)

                btot = vp.tile([128, 8], F32, tag="btot", name="btot")
                for m in range(8):
                    ps = pm.tile([128, 2], F32, tag="ps", name="ps")
                    mmv(ps[:], w1[:, 128 * m:128 * m + 128], bv[:, 0:2],
                        start=True, stop=False)
                    mmv(ps[:], w1[:, 1024 + 128 * m:1024 + 128 * m + 128],
                        bv[:, 2:4], start=False, stop=True)
                    V.tensor_scalar(btot[:, m:m + 1], ps[:, 0:1], tb1[:, m:m + 1],
                                    None, ALU.add)
                for d in range(2):
                    V.tensor_scalar(w1[:, 1024 * d:1024 * d + 1024],
                                    w1[:, 1024 * d:1024 * d + 1024],
                                    av[:, d:d + 1], None, ALU.mult)

                for gc, (o, w) in enumerate(CHUNKS):
                    nsb = (w + 127) // 128
                    mub = pm.tile([128, 512], F32, tag="ps", name="mub")
                    rb = pm.tile([128, 512], F32, tag="ps", name="rb")
                    for s in range(nsb):
                        mc0 = 512 * gc + 128 * s
                        T.matmul(mub[:, 128 * s:128 * s + min(128, w - 128 * s)],
                                 sel[:, 0:128],
                                 murT[0:2, mc0:mc0 + min(128, w - 128 * s)],
                                 start=True, stop=True)
                        T.matmul(rb[:, 128 * s:128 * s + min(128, w - 128 * s)],
                                 sel[:, 128:256],
                                 murT[0:2, mc0:mc0 + min(128, w - 128 * s)],
                                 start=True, stop=True)
                    xh = []
                    for dt in range(2):
                        x_ = cp.tile([128, 512], F32R, tag=f"big{dt}", name=f"xh{dt}")
                        V.tensor_tensor(x_[:, 0:w], tokT[dt][:, o:o + w],
                                        mub[:, 0:w], ALU.subtract)
                        V.tensor_tensor(x_[:, 0:w], x_[:, 0:w], rb[:, 0:w],
                                        ALU.mult)
                        xh.append(x_)
                    A = cp.tile([128, 8 * 512], F32R, tag="A", name="A", bufs=1)
                    for m in range(8):
                        hp = ph1.tile([128, 512], F32, tag="h1", name="h1")
                        T.matmul(hp[:, 0:w], w1[:, 128 * m:128 * m + 128],
                                 xh[0][:, 0:w], start=True, stop=False)
                        T.matmul(hp[:, 0:w], w1[:, 1024 + 128 * m:1024 + 128 * m + 128],
                                 xh[1][:, 0:w], start=False, stop=True)
                        S.activation(A[:, 512 * m:512 * m + w], hp[:, 0:w],
                                     AF.Gelu, bias=btot[:, m:m + 1])
                    for d in range(2):
                        h2p = ph2.tile([128, 512], F32, tag="h2", name="h2")
                        for k in range(8):
                            T.matmul(h2p[:, 0:w],
                                     w2[:, 256 * k + 128 * d:256 * k + 128 * d + 128],
                                     A[:, 512 * k:512 * k + w],
                                     start=(k == 0), stop=False)
                        T.matmul(h2p[:, 0:w], b2row[0:1, 128 * d:128 * d + 128],
                                 onesr[0:1, 0:w], start=False, stop=True)
                        gcol = 4 + d
                        V.scalar_tensor_tensor(tokT[d][:, o:o + w], h2p[:, 0:w],
                                               mvec[:, gcol:gcol + 1],
                                               tokT[d][:, o:o + w],
                                               ALU.mult, ALU.add)

            fing = vp.tile([128, 2], F32, tag="fing", name="fing")
            nc.sync.dma_start(fing[:], col2(d_fing, 2))
            finb = vp.tile([128, 2], F32, tag="finb", name="finb")
            nc.sync.dma_start(finb[:], col2(d_finb, 2))
            outw = vp.tile([128, 4], F32R, tag="outw", name="outw")
            ld_split(outw[:], d_outwT, 2)
            outbs = vp.tile([C, 1], F32, tag="outbs", name="outbs")
            nc.sync.dma_start(outbs[:], d_outb[:, :])
            wpr = vp.tile([128, 4], F32R, tag="wpr", name="wpr")
            vb = vp.tile([128, 4], F32R, tag="vb", name="vb")
            for dt in range(2):
                V.tensor_scalar(wpr[:, 2 * dt:2 * dt + 2],
                                outw[:, 2 * dt:2 * dt + 2],
                                fing[:, dt:dt + 1], None, ALU.mult)
                V.tensor_scalar(vb[:, 2 * dt:2 * dt + 2],
                                outw[:, 2 * dt:2 * dt + 2],
                                finb[:, dt:dt + 1], None, ALU.mult)
            pw = pm.tile([2, 4], F32, tag="ps", name="pw")
            for dt in range(2):
                mmv(pw[0:2, 0:2], wpr[:, 2 * dt:2 * dt + 2], onescol[:],
                    start=(dt == 0), stop=(dt == 1))
            for dt in range(2):
                mmv(pw[0:2, 2:4], vb[:, 2 * dt:2 * dt + 2], onescol[:],
                    start=(dt == 0), stop=(dt == 1))
            nws = vp.tile([2, 2], F32, tag="nws", name="nws")
            V.tensor_scalar(nws[0:2, 0:1], pw[0:2, 0:1], -1.0, None, ALU.mult)
            V.tensor_tensor(nws[0:2, 1:2], pw[0:2, 2:3], outbs[:, 0:1], ALU.add)

            for ci, (o, w) in enumerate(LCH):
                slot = tokL[ci % NTOKL]
                for dt in range(2):
                    tpp = pm.tile([128, 128], F32, tag="ps", name="tpp")
                    T.transpose(tpp[0:w, 0:128],
                                tokT[dt][:, o:o + w].bitcast(F32),
                                id128[:, :].bitcast(F32))
                    S.copy(slot[0:w, 128 * dt:128 * dt + 128], tpp[0:w, 0:128])
                V.tensor_reduce(muS[:, ci:ci + 1], slot[:, 0:256],
                                mybir.AxisListType.X, ALU.add)
                scrq = cp.tile([128, 256], F32R, tag="scr2", name="scrq")
                S.activation(scrq[:], slot[:, 0:256], AF.Square,
                             accum_out=m2S[:, ci:ci + 1])
            V.tensor_scalar(muS[:, 0:NL], muS[:, 0:NL], 1.0 / 256,
                            None, ALU.mult)
            V.tensor_scalar(m2S[:, 0:NL], m2S[:, 0:NL], 1.0 / 256,
                            None, ALU.mult)
            musq = vp.tile([128, 48], F32, tag="musq", name="musq")
            V.tensor_tensor(musq[:, 0:NL], muS[:, 0:NL], muS[:, 0:NL], ALU.mult)
            var48 = vp.tile([128, 48], F32, tag="var48", name="var48")
            V.scalar_tensor_tensor(var48[:, 0:NL], m2S[:, 0:NL], EPS,
                                   musq[:, 0:NL], ALU.add, ALU.subtract)
            V.reciprocal(var48[:, 0:NL], var48[:, 0:NL])
            st2v = stat2[:].rearrange("p (ci two) -> p two ci", two=2)
            V.tensor_copy(st2v[:, 0, :], muS[:, 0:NL])
            S.activation(st2v[:, 1, :], var48[:, 0:NL], AF.Sqrt)

            for g, (o5, w5) in enumerate(CHUNKS):
                ns = (w5 + 127) // 128
                mt = pm.tile([2, 512], F32, tag="ps", name="mt")
                for s in range(ns):
                    ci = 4 * g + s
                    T.transpose(mt[0:2, 128 * s:128 * s + 128],
                                stat2[:, 2 * ci:2 * ci + 2].bitcast(F32),
                                id128[:, :].bitcast(F32))
                mts = vp.tile([2, 512], F32R, tag="mts", name="mts")
                V.tensor_copy(mts[0:2, 0:128 * ns], mt[0:2, 0:128 * ns])
                z_ps = pm.tile([2, 512], F32, tag="ps", name="z_ps")
                for dt in range(2):
                    T.matmul(z_ps[0:2, 0:w5], wpr[:, 2 * dt:2 * dt + 2],
                             tokT[dt][:, o5:o5 + w5],
                             start=(dt == 0), stop=(dt == 1))
                mr2 = pm.tile([2, 512], F32, tag="ps", name="mr2")
                T.matmul(mr2[0:2, 0:w5], sel[0:2, 0:2], mts[0:2, 0:w5],
                         start=True, stop=True)
                rr2 = pm.tile([2, 512], F32, tag="ps", name="rr2")
                T.matmul(rr2[0:2, 0:w5], sel[0:2, 128:130],
                         mts[0:2, 0:w5], start=True, stop=True)
                zc = vp.tile([2, 512], F32, tag="zc", name="zc")
                V.tensor_copy(zc[0:2, 0:w5], z_ps[0:2, 0:w5])
                t1 = vp.tile([2, 512], F32, tag="t1", name="t1")
                V.scalar_tensor_tensor(t1[0:2, 0:w5], mr2[0:2, 0:w5],
                                       nws[0:2, 0:1], zc[0:2, 0:w5],
                                       ALU.mult, ALU.add)
                ot = cp.tile([C, 512], F32, tag="osb", name="osb", bufs=1)
                V.tensor_tensor(ot[0:2, 0:w5], t1[0:2, 0:w5],
                                rr2[0:2, 0:w5], ALU.mult)
                V.tensor_scalar(ot[0:2, 0:w5], ot[0:2, 0:w5], nws[0:2, 1:2],
                                None, ALU.add)
                nc.sync.dma_start(d_outT[:, o5:o5 + w5], ot[:, 0:w5])

    split_excess_waits(nc)
    return nc


_NC_CACHE = {}


def _get_nc(depth=DEPTH):
    key = depth
    if key not in _NC_CACHE:
        _NC_CACHE[key] = build_nc(depth)
    return _NC_CACHE[key]


def _freqs_hilo():
    f32 = np.float32
    fr = np.exp(
        -np.log(10000.0) * np.arange(TE // 2, dtype=f32) / (TE // 2)
    ).astype(f32)
    hi = (fr.view(np.uint32) & np.uint32(0xFFFFF000)).view(f32)
    lo = (fr - hi).astype(f32)
    return np.stack([hi, lo], axis=1).astype(f32)


def _shared_inputs(inputs):
    f32 = np.float32
    sh = {
        "posT": np.ascontiguousarray(inputs["pos"][0].T.astype(f32)),
        "inwT": np.ascontiguousarray(inputs["in_w"].T.astype(f32)),
        "inb": inputs["in_b"].reshape(D, 1).astype(f32),
        "freqs": _freqs_hilo(),
        "tp1T": np.ascontiguousarray(inputs["tp1_w"].T.astype(f32)),
        "tp1b": inputs["tp1_b"].reshape(D, 1).astype(f32),
        "tp2T": np.ascontiguousarray(inputs["tp2_w"].T.astype(f32)),
        "tp2b": inputs["tp2_b"].reshape(D, 1).astype(f32),
        "clsv": inputs["cls_tok"].reshape(D, 1).astype(f32),
        "qkvoT": np.ascontiguousarray(
            np.stack(
                [
                    np.stack(
                        [
                            inputs["attn_in_w"][i][0:D].T,
                            inputs["attn_in_w"][i][D:2 * D],
                            inputs["attn_in_w"][i][2 * D:3 * D].T,
                            inputs["attn_out_w"][i].T,
                        ]
                    )
                    for i in range(DEPTH)
                ]
            ).astype(f32)
        ),
        "attnb": np.ascontiguousarray(
            np.stack(
                [
                    np.stack(
                        [
                            inputs["attn_in_b"][i][0:D],
                            inputs["attn_in_b"][i][D:2 * D],
                            inputs["attn_in_b"][i][2 * D:3 * D],
                            inputs["attn_out_b"][i],
                        ]
                    )
                    for i in range(DEPTH)
                ]
            ).astype(f32).reshape(DEPTH, 4, D, 1)
        ),
        "modT": np.ascontiguousarray(
            np.transpose(inputs["adaln_mod_w"], (0, 1, 3, 2)).astype(f32)
        ),
        "modb": inputs["adaln_mod_b"].astype(f32).reshape(DEPTH, 3, 3 * D, 1),
        "lng": inputs["adaln_ln_g"].astype(f32).reshape(DEPTH, 3, D, 1),
        "lnb": inputs["adaln_ln_b"].astype(f32).reshape(DEPTH, 3, D, 1),
        "w1T": np.ascontiguousarray(
            np.transpose(inputs["mlp_w1"], (0, 1, 3, 2)).astype(f32)
        ),
        "b1": inputs["mlp_b1"].astype(f32).reshape(DEPTH, 2, FF, 1),
        "w2T": np.ascontiguousarray(
            np.transpose(inputs["mlp_w2"], (0, 1, 3, 2)).astype(f32)
        ),
        "b2": inputs["mlp_b2"].astype(f32).reshape(DEPTH, 2, D, 1),
        "fing": inputs["fin_g"].reshape(D, 1).astype(f32),
        "finb": inputs["fin_b"].reshape(D, 1).astype(f32),
        "outwT": np.ascontiguousarray(inputs["out_w"].T.astype(f32)),
        "outb": inputs["out_b"].reshape(C, 1).astype(f32),
        "ident": np.eye(8, dtype=f32),
        "ident128": np.eye(128, dtype=f32),
        "onessc": np.full((128, 1), 1.0 / 256, dtype=f32),
        "onesw": np.ones((128, 512), f32),
        "zerosw": np.zeros((128, NL * 8), f32),
        "selw": np.concatenate(
            [np.tile(np.array([[1.0], [0.0]], f32), (1, 128)),
             np.tile(np.array([[0.0], [1.0]], f32), (1, 128))], axis=1),
    }
    return sh


def kernel(**inputs):
    global LAST
    nc = _get_nc()
    sh = _shared_inputs(inputs)
    x_t = np.asarray(inputs["x_t"], dtype=np.float32)
    tv = np.asarray(inputs["t"]).astype(np.int32)
    in_maps = []
    for c in range(NCORES):
        m = dict(sh)
        m["xT"] = np.ascontiguousarray(x_t[c].T)
        m["tval"] = tv[c].reshape(1, 1)
        in_maps.append(m)
    res = run_bass_kernel_spmd(
        nc, in_maps, core_ids=list(range(NCORES)), trace=TRACE
    )
    LAST = res
    out = np.stack(
        [np.ascontiguousarray(res.results[c]["outT"].T) for c in range(NCORES)]
    ).astype(np.float32)
    return out



# revision 12
# speedup vs baseline: 1.1405x; 1.1405x over previous
import sys

for _p in ("/opt/trn_rl_repo", "/opt/pypackages"):
    if _p not in sys.path:
        sys.path.append(_p)

import numpy as np
import ml_dtypes

_BF16NP = ml_dtypes.bfloat16
import concourse.bass as bass
import concourse.tile as tile
from concourse import mybir
from concourse.bass_utils import run_bass_kernel_spmd

AF = mybir.ActivationFunctionType
ALU = mybir.AluOpType
F32R = mybir.dt.float32r
F32 = mybir.dt.float32
BF16 = mybir.dt.bfloat16
I32 = mybir.dt.int32

B, L, C, D, H, DEPTH, FF, TE = 8, 5160, 2, 256, 8, 8, 1024, 256
HD = D // H
NCORES = 8
EPS = 1e-5
PI = float(np.pi)
ISQ = float(1.0 / np.sqrt(HD))

CHUNKS = [(i * 512, 512) for i in range(10)] + [(5120, 40)]
LCH = [(i * 128, 128) for i in range(40)] + [(5120, 40)]
NL = len(LCH)

TRACE = False
LAST = None


def split_excess_waits(nc, limit=1):
    fn = nc.m.functions[0]
    blocks = getattr(fn, "instruction_blocks", None) or getattr(fn, "blocks")
    for bb in blocks:
        insts = bb.instructions
        out = []
        for inst in insts:
            si = inst.sync_info
            waits = list(si.on_wait) if si is not None and si.on_wait else []
            if len(waits) > limit:
                keep = waits[-limit:]
                excess = waits[:-limit]
                for i in range(0, len(excess), limit):
                    nop = mybir.InstNoOp(
                        name=nc.get_next_instruction_name(),
                        sync_info=mybir.SyncInfo(
                            on_wait=excess[i:i + limit], on_update=[]
                        ),
                        bass_nofuse=True,
                        engine=inst.engine,
                    )
                    nc.register_instruction(nop)
                    out.append(nop)
                si.on_wait = keep
            out.append(inst)
        if len(out) != len(insts):
            insts[:] = out
    return nc


def build_nc(depth=DEPTH):
    nc = bass.Bass(target_bir_lowering=False, trn_type="TRN2")
    V = nc.vector
    S = nc.scalar
    G = nc.gpsimd
    T = nc.tensor

    def mmv(out, lhsT, rhs2, start, stop):
        T.matmul(out, lhsT, rhs2, start=start, stop=stop)

    d_xT = nc.dram_tensor("xT", [C, L], F32R, kind="ExternalInput")
    d_t = nc.dram_tensor("tval", [1, 1], I32, kind="ExternalInput")
    d_posT = nc.dram_tensor("posT", [D, L], F32, kind="ExternalInput")
    d_inwT = nc.dram_tensor("inwT", [C, D], F32R, kind="ExternalInput")
    d_inb = nc.dram_tensor("inb", [D, 1], F32, kind="ExternalInput")
    d_freqs = nc.dram_tensor("freqs", [TE // 2, 2], F32, kind="ExternalInput")
    d_tp1T = nc.dram_tensor("tp1T", [TE, D], F32R, kind="ExternalInput")
    d_tp1b = nc.dram_tensor("tp1b", [D, 1], F32, kind="ExternalInput")
    d_tp2T = nc.dram_tensor("tp2T", [D, D], F32R, kind="ExternalInput")
    d_tp2b = nc.dram_tensor("tp2b", [D, 1], F32, kind="ExternalInput")
    d_cls = nc.dram_tensor("clsv", [D, 1], F32R, kind="ExternalInput")
    d_qkvoT = nc.dram_tensor("qkvoT", [DEPTH, 4, D, D], F32R, kind="ExternalInput")
    d_attnb = nc.dram_tensor("attnb", [DEPTH, 4, D, 1], F32, kind="ExternalInput")
    d_modT = nc.dram_tensor("modT", [DEPTH, 3, D, 3 * D], F32R, kind="ExternalInput")
    d_mod01T = nc.dram_tensor("mod01T", [DEPTH, 2, D, 3 * D], BF16, kind="ExternalInput")
    d_modb = nc.dram_tensor("modb", [DEPTH, 3, 3 * D, 1], F32, kind="ExternalInput")
    d_modb01 = nc.dram_tensor("modb01", [128, 96], F32, kind="ExternalInput")
    d_lng = nc.dram_tensor("lng", [DEPTH, 3, D, 1], F32, kind="ExternalInput")
    d_lnb = nc.dram_tensor("lnb", [DEPTH, 3, D, 1], F32, kind="ExternalInput")
    d_w1T = nc.dram_tensor("w1T", [DEPTH, 2, D, FF], F32R, kind="ExternalInput")
    d_b1 = nc.dram_tensor("b1", [DEPTH, 2, FF, 1], F32, kind="ExternalInput")
    d_w2T = nc.dram_tensor("w2T", [DEPTH, 2, FF, D], F32R, kind="ExternalInput")
    d_w2Th = nc.dram_tensor("w2Th", [FF, DEPTH * D], BF16, kind="ExternalInput")
    d_b2 = nc.dram_tensor("b2", [DEPTH, 2, D, 1], F32, kind="ExternalInput")
    d_fing = nc.dram_tensor("fing", [D, 1], F32, kind="ExternalInput")
    d_finb = nc.dram_tensor("finb", [D, 1], F32, kind="ExternalInput")
    d_outwT = nc.dram_tensor("outwT", [D, C], F32R, kind="ExternalInput")
    d_outb = nc.dram_tensor("outb", [C, 1], F32, kind="ExternalInput")
    d_ident = nc.dram_tensor("ident", [8, 8], F32R, kind="ExternalInput")
    d_ident128 = nc.dram_tensor("ident128", [128, 128], F32R, kind="ExternalInput")
    d_sel = nc.dram_tensor("selw", [2, 256], F32R, kind="ExternalInput")
    d_ones = nc.dram_tensor("onesw", [128, 512], F32R, kind="ExternalInput")
    d_onessc = nc.dram_tensor("onessc", [128, 1], F32R, kind="ExternalInput")
    d_onesb = nc.dram_tensor("onesb", [128, 2], BF16, kind="ExternalInput")
    d_outT = nc.dram_tensor("outT", [C, L], F32, kind="ExternalOutput")

    def col2(dram_ap, groups):
        return dram_ap[:, 0].rearrange("(g p) -> p g", p=128)

    def ld_split(dst, dram2d, g):
        x = dram2d.shape[1]
        nc.sync.dma_start(
            dst.rearrange("p (g x) -> p g x", g=g),
            dram2d.rearrange("(g p) x -> p g x", p=128))

    with tile.TileContext(nc) as tc:
        with tc.tile_pool(name="state", bufs=1) as st, \
             tc.tile_pool(name="wts", bufs=2) as wp, \
             tc.tile_pool(name="vecs", bufs=2) as vp, \
             tc.tile_pool(name="chk", bufs=2) as cp, \
             tc.tile_pool(name="ph1", bufs=2, space="PSUM") as ph1, \
             tc.tile_pool(name="ph2", bufs=2, space="PSUM") as ph2, \
             tc.tile_pool(name="py", bufs=1, space="PSUM") as pyp, \
             tc.tile_pool(name="pmisc", bufs=3, space="PSUM") as pm:

            tokT = [st.tile([128, L], F32R, tag=f"tok{d}", name=f"tok{d}") for d in range(2)]
            pT = st.tile([128, NL * 8], BF16, tag="pT", name="pT")
            stat2 = st.tile([128, 2 * NL], F32, tag="stat2", name="stat2")
            murT = st.tile([2, NL * 128], F32R, tag="murT", name="murT")
            sel = st.tile([2, 256], F32R, tag="sel", name="sel")
            onesr = st.tile([1, 512], F32R, tag="onesr", name="onesr")
            onescol = st.tile([128, 2], F32R, tag="onescol", name="onescol")
            cls = [st.tile([128, 1], F32R, tag=f"cls{d}", name=f"cls{d}") for d in range(2)]
            temb = [st.tile([128, 2], F32R, tag=f"temb{d}", name=f"temb{d}") for d in range(2)]
            stm = [st.tile([128, 2], F32R, tag=f"stm{d}", name=f"stm{d}") for d in range(2)]
            modpre = st.tile([128, DEPTH * 12], F32, tag="modpre", name="modpre")
            abpre = st.tile([128, DEPTH * 8], F32, tag="abpre", name="abpre")
            identsb = st.tile([8, 8], F32R, tag="ident", name="ident")
            id128 = st.tile([128, 128], F32R, tag="id128", name="id128")
            onessc = st.tile([128, 1], F32R, tag="onessc", name="onessc")
            epsc = st.tile([128, 1], F32, tag="epsc", name="epsc")
            lngsb = st.tile([128, DEPTH * 6], F32, tag="lngsb", name="lngsb")
            lnbsb = st.tile([128, DEPTH * 6], F32, tag="lnbsb", name="lnbsb")
            NTOKL = 4
            tokL = [st.tile([128, 264], BF16, tag=f"tokL{j}", name=f"tokL{j}")
                    for j in range(NTOKL)]

            nc.sync.dma_start(identsb[:], d_ident[:, :])
            nc.sync.dma_start(id128[:], d_ident128[:, :])
            nc.sync.dma_start(onessc[:], d_onessc[:, :])
            nc.sync.dma_start(onesr[:], d_ones[0:1, :])
            nc.sync.dma_start(onescol[:], d_ones[:, 0:2])
            V.memset(pT[:], 0.0)
            V.memset(epsc[:], EPS)
            V.memset(stat2[:], 1.0)
            nc.sync.dma_start(sel[:], d_sel[:, :])
            for j in range(NTOKL):
                nc.sync.dma_start(tokL[j][:, 256:258], d_onesb[:, :])
            for dt in range(2):
                nc.sync.dma_start(
                    lngsb[:].rearrange("p (i g dt) -> p i g dt",
                                       i=DEPTH, g=3)[:, :, :, dt],
                    d_lng[:, :, 128 * dt:128 * dt + 128, 0].rearrange(
                        "i g p -> p i g"))
                nc.sync.dma_start(
                    lnbsb[:].rearrange("p (i g dt) -> p i g dt",
                                       i=DEPTH, g=3)[:, :, :, dt],
                    d_lnb[:, :, 128 * dt:128 * dt + 128, 0].rearrange(
                        "i g p -> p i g"))
            nc.sync.dma_start(cls[0][:], d_cls[0:128, :])
            nc.sync.dma_start(cls[1][:], d_cls[128:256, :])

            with tc.tile_pool(name="pre", bufs=1) as pre:
                tfl = pre.tile([1, 2], F32R, tag="tfl", name="tfl")
                traw = pre.tile([1, 1], I32, tag="traw", name="traw")
                nc.sync.dma_start(traw[:], d_t[:, :])
                V.tensor_copy(tfl[:], traw[:].to_broadcast([1, 2]))
                tb = pm.tile([128, 2], F32, tag="ps", name="tb")
                mmv(tb[:], onesr[0:1, 0:128], tfl[:], start=True, stop=True)
                fsb = pre.tile([128, 2], F32, tag="fsb", name="fsb")
                nc.sync.dma_start(fsb[:], d_freqs[:, :])
                ang = pre.tile([128, 1], F32, tag="ang", name="ang")
                ang2 = pre.tile([128, 1], F32, tag="ang2", name="ang2")
                V.tensor_tensor(ang[:], tb[:, 0:1], fsb[:, 0:1], ALU.mult)
                V.tensor_tensor(ang2[:], tb[:, 0:1], fsb[:, 1:2], ALU.mult)
                V.tensor_tensor(ang[:], ang[:], ang2[:], ALU.add)
                dsc = pre.tile([128, 1], F32, tag="dsc", name="dsc")
                qi = pre.tile([128, 1], I32, tag="qi", name="qi")
                qf = pre.tile([128, 1], F32, tag="qf", name="qf")
                msk = pre.tile([128, 1], F32, tag="msk", name="msk")
                TWO_PI = 2 * PI

                def mod2pi(dst, shift):
                    V.tensor_scalar(dst[:], ang[:], shift, None, ALU.add)
                    V.tensor_scalar(dsc[:], dst[:], 1.0 / TWO_PI, 0.5,
                                    ALU.mult, ALU.subtract)
                    V.tensor_copy(qi[:], dsc[:])
                    V.tensor_copy(qf[:], qi[:])
                    V.scalar_tensor_tensor(dst[:], qf[:], -TWO_PI, dst[:],
                                           ALU.mult, ALU.add)
                    V.tensor_scalar(msk[:], dst[:], TWO_PI, None, ALU.is_ge)
                    V.scalar_tensor_tensor(dst[:], msk[:], -TWO_PI, dst[:],
                                           ALU.mult, ALU.add)
                    V.tensor_scalar(msk[:], dst[:], 0.0, None, ALU.is_lt)
                    V.scalar_tensor_tensor(dst[:], msk[:], TWO_PI, dst[:],
                                           ALU.mult, ALU.add)
                    V.tensor_scalar(dst[:], dst[:], PI, None, ALU.subtract)

                m1 = pre.tile([128, 1], F32, tag="m1", name="m1")
                mod2pi(m1, PI)
                m2 = pre.tile([128, 1], F32, tag="m2", name="m2")
                mod2pi(m2, 1.5 * PI)
                sinf = pre.tile([128, 2], F32R, tag="sinf", name="sinf")
                cosf = pre.tile([128, 2], F32R, tag="cosf", name="cosf")
                S.activation(sinf[:], m1[:].to_broadcast([128, 2]), AF.Sin)
                S.activation(cosf[:], m2[:].to_broadcast([128, 2]), AF.Sin)

                ttp1 = pre.tile([128, 512], F32R, tag="ttp1", name="ttp1")
                ld_split(ttp1[:], d_tp1T, 2)
                ttp2 = pre.tile([128, 512], F32R, tag="ttp2", name="ttp2")
                ld_split(ttp2[:], d_tp2T, 2)
                tp1b = pre.tile([128, 2], F32, tag="tp1b", name="tp1b")
                nc.sync.dma_start(tp1b[:], col2(d_tp1b, 2))
                tp2b = pre.tile([128, 2], F32, tag="tp2b", name="tp2b")
                nc.sync.dma_start(tp2b[:], col2(d_tp2b, 2))

                st1 = [pre.tile([128, 2], F32R, tag=f"st1{m}", name=f"st1{m}") for m in range(2)]
                for m in range(2):
                    ps = pm.tile([128, 2], F32, tag="ps", name="ps")
                    mmv(ps[:], ttp1[:, 128 * m:128 * m + 128], sinf[:],
                        start=True, stop=False)
                    mmv(ps[:], ttp1[:, 256 + 128 * m:256 + 128 * m + 128],
                        cosf[:], start=False, stop=True)
                    S.activation(st1[m][:], ps[:], AF.Silu, bias=tp1b[:, m:m + 1])
                for m in range(2):
                    ps = pm.tile([128, 2], F32, tag="ps", name="ps")
                    mmv(ps[:], ttp2[:, 128 * m:128 * m + 128], st1[0][:],
                        start=True, stop=False)
                    mmv(ps[:], ttp2[:, 256 + 128 * m:256 + 128 * m + 128],
                        st1[1][:], start=False, stop=True)
                    S.activation(temb[m][:], ps[:], AF.Identity,
                                 bias=tp2b[:, m:m + 1])
                    S.activation(stm[m][:], temb[m][:], AF.Silu)

            with tc.tile_pool(name="pre2", bufs=1) as pre:
                inwsb = pre.tile([C, D], F32R, tag="inwsb", name="inwsb")
                nc.sync.dma_start(inwsb[:], d_inwT[:, :])
                inbsb = pre.tile([128, 2], F32, tag="inbsb", name="inbsb")
                nc.sync.dma_start(inbsb[:], col2(d_inb, 2))
                for (o, w) in CHUNKS:
                    xtc = pre.tile([C, 512], F32R, tag="xtc", name="xtc")
                    nc.sync.dma_start(xtc[:, 0:w], d_xT[:, o:o + w])
                    for dt in range(2):
                        ppc = pre.tile([128, 512], F32, tag=f"ppc{dt}", name=f"ppc{dt}")
                        nc.sync.dma_start(ppc[:, 0:w],
                                          d_posT[128 * dt:128 * dt + 128, o:o + w])
                        ps = pm.tile([128, 512], F32, tag="ps", name="ps")
                        T.matmul(ps[:, 0:w], inwsb[:, 128 * dt:128 * dt + 128],
                                 xtc[:, 0:w], start=True, stop=True)
                        V.scalar_tensor_tensor(
                            tokT[dt][:, o:o + w], ps[:, 0:w],
                            inbsb[:, dt:dt + 1], ppc[:, 0:w], ALU.add, ALU.add)

            with tc.tile_pool(name="pre3", bufs=2) as pre:
                for i in range(depth):
                    for g in range(2):
                        tmg = pre.tile([128, 1536], F32R, tag="tmg", name="tmg")
                        ld_split(tmg[:], d_modT[i, g], 2)
                        tmb = pre.tile([128, 6], F32, tag="tmb", name="tmb")
                        nc.sync.dma_start(tmb[:], col2(d_modb[i, g], 6))
                        for m in range(6):
                            ps = pm.tile([128, 2], F32, tag="ps", name="ps")
                            mmv(ps[:], tmg[:, 128 * m:128 * m + 128],
                                stm[0][:], start=True, stop=False)
                            mmv(ps[:], tmg[:, 768 + 128 * m:768 + 128 * m + 128],
                                stm[1][:], start=False, stop=True)
                            colm = i * 12 + g * 6 + m
                            V.tensor_scalar(modpre[:, colm:colm + 1], ps[:, 0:1],
                                            tmb[:, m:m + 1], None, ALU.add)
                        scr = pre.tile([128, 1], F32, tag="scr", name="scr", bufs=1)
                        for dt in range(2):
                            scol = i * 12 + g * 6 + dt
                            shcol = i * 12 + g * 6 + 2 + dt
                            lcol = i * 6 + g * 2 + dt
                            acol = i * 8 + g * 4 + dt
                            bcol = i * 8 + g * 4 + 2 + dt
                            V.tensor_scalar(scr[:], modpre[:, scol:scol + 1],
                                            1.0, None, ALU.add)
                            V.tensor_tensor(abpre[:, acol:acol + 1], scr[:],
                                            lngsb[:, lcol:lcol + 1], ALU.mult)
                            V.tensor_tensor(abpre[:, bcol:bcol + 1], scr[:],
                                            lnbsb[:, lcol:lcol + 1], ALU.mult)
                            V.tensor_tensor(abpre[:, bcol:bcol + 1],
                                            abpre[:, bcol:bcol + 1],
                                            modpre[:, shcol:shcol + 1], ALU.add)

            def cls_ln(i, g, out_tag):
                csc = [vp.tile([128, 2], F32R, tag=f"csc{d}", name=f"csc{d}") for d in range(2)]
                for d in range(2):
                    V.tensor_copy(csc[d][:, 0:1], onessc[:, 0:1])
                    V.tensor_scalar(csc[d][:, 1:2], cls[d][:], 1.0 / 256, None,
                                    ALU.mult)
                ps = pm.tile([1, 2], F32, tag="ps", name="ps")
                for d in range(2):
                    mmv(ps[0:1, 0:2], cls[d][:], csc[d][:, 0:2],
                        start=(d == 0), stop=(d == 1))
                mc = vp.tile([1, 8], F32R, tag="mc", name="mc")
                V.tensor_copy(mc[0:1, 0:2], ps[0:1, 0:2])
                V.tensor_tensor(mc[0:1, 2:3], mc[0:1, 0:1], mc[0:1, 0:1],
                                ALU.mult)
                V.scalar_tensor_tensor(mc[0:1, 3:4], mc[0:1, 1:2], EPS,
                                       mc[0:1, 2:3], ALU.add, ALU.subtract)
                S.activation(mc[0:1, 3:4], mc[0:1, 3:4], AF.Ln)
                S.activation(mc[0:1, 3:4], mc[0:1, 3:4], AF.Exp, scale=-0.5)
                V.tensor_copy(mc[0:1, 4:6], mc[0:1, 0:1].to_broadcast([1, 2]))
                V.tensor_copy(mc[0:1, 6:8], mc[0:1, 3:4].to_broadcast([1, 2]))
                mcb = pm.tile([128, 2], F32, tag="ps", name="mcb")
                rcb = pm.tile([128, 2], F32, tag="ps", name="rcb")
                mmv(mcb[:], onesr[0:1, 0:128], mc[0:1, 4:6],
                    start=True, stop=True)
                mmv(rcb[:], onesr[0:1, 0:128], mc[0:1, 6:8],
                    start=True, stop=True)
                hc = [vp.tile([128, 2], F32R, tag=f"{out_tag}{d}", name=f"{out_tag}{d}") for d in range(2)]
                for d in range(2):
                    acol = i * 8 + g * 4 + d
                    bcol = i * 8 + g * 4 + 2 + d
                    V.tensor_tensor(hc[d][:], cls[d][:].to_broadcast([128, 2]),
                                    mcb[:], ALU.subtract)
                    V.tensor_tensor(hc[d][:], hc[d][:], rcb[:], ALU.mult)
                    V.scalar_tensor_tensor(
                        hc[d][:], hc[d][:], abpre[:, acol:acol + 1],
                        abpre[:, bcol:bcol + 1].to_broadcast([128, 2]),
                        ALU.mult, ALU.add)
                return hc

            for i in range(depth):
                qkvo = wp.tile([128, 2048], F32R, tag="qkvo", name="qkvo")
                for dt in range(2):
                    nc.sync.dma_start(
                        qkvo[:, 1024 * dt:1024 * dt + 1024].rearrange(
                            "p (w x) -> p w x", w=4),
                        d_qkvoT[i][:, 128 * dt:128 * dt + 128, :].rearrange(
                            "w p x -> p w x"))
                w1 = wp.tile([128, 2048], F32R, tag="w1", name="w1")
                ld_split(w1[:], d_w1T[i, 1], 2)
                w2 = wp.tile([128, 2048], BF16, tag="w2", name="w2")
                ld_split(w2[:], d_w2Th[:, D * i:D * i + D], 8)
                w1c = wp.tile([128, 2048], F32R, tag="w1c", name="w1c", bufs=1)
                ld_split(w1c[:], d_w1T[i, 0], 2)
                w2c = wp.tile([128, 2048], F32R, tag="w2c", name="w2c", bufs=1)
                ld_split(w2c[:], d_w2T[i, 0], 8)
                mod2 = wp.tile([128, 1536], F32R, tag="mod2", name="mod2", bufs=1)
                ld_split(mod2[:], d_modT[i, 2], 2)
                tattnb = vp.tile([128, 8], F32, tag="tattnb", name="tattnb")
                nc.sync.dma_start(
                    tattnb[:].rearrange("p (w dt) -> p w dt", w=4),
                    d_attnb[i][:, :, 0].rearrange("w (dt p) -> p w dt", p=128))
                tb1 = vp.tile([128, 8], F32, tag="tb1", name="tb1")
                nc.sync.dma_start(tb1[:], col2(d_b1[i, 1], 8))
                tb1c = vp.tile([128, 8], F32, tag="tb1c", name="tb1c")
                nc.sync.dma_start(tb1c[:], col2(d_b1[i, 0], 8))
                b2row = vp.tile([1, 256], F32R, tag="b2row", name="b2row", bufs=1)
                nc.sync.dma_start(b2row[:], d_b2[i, 1].rearrange("d o -> o d").bitcast(F32R))
                tb2c = vp.tile([128, 2], F32, tag="tb2c", name="tb2c")
                nc.sync.dma_start(tb2c[:], col2(d_b2[i, 0], 2))
                tmodb2 = vp.tile([128, 6], F32, tag="tmodb2", name="tmodb2")
                nc.sync.dma_start(tmodb2[:], col2(d_modb[i, 2], 6))

                hc = cls_ln(i, 0, "hca")
                Qm = [vp.tile([128, 8], F32R, tag=f"qm{d}", name=f"qm{d}") for d in range(2)]
                for d in range(2):
                    qp = pm.tile([128, 2], F32, tag="ps", name="ps")
                    mmv(qp[:], qkvo[:, 128 * d:128 * d + 128],
                        hc[0][:], start=True, stop=False)
                    mmv(qp[:], qkvo[:, 1024 + 128 * d:1024 + 128 * d + 128],
                        hc[1][:], start=False, stop=True)
                    V.memset(Qm[d][:].bitcast(F32), 0.0)
                    for hh in range(4):
                        r0 = 32 * hh
                        col = 4 * d + hh
                        V.tensor_scalar(Qm[d][r0:r0 + 32, col:col + 1],
                                        qp[r0:r0 + 32, 0:1],
                                        tattnb[r0:r0 + 32, 0 + d:d + 1],
                                        None, ALU.add)
                wq = [vp.tile([128, 8], F32R, tag=f"wq{d}", name=f"wq{d}") for d in range(2)]
                for cb in range(2):
                    wqp = pm.tile([128, 8], F32, tag="ps", name="ps")
                    for fb in range(2):
                        T.matmul(wqp[:],
                                 qkvo[:, 1024 * fb + 256 + 128 * cb:
                                      1024 * fb + 256 + 128 * cb + 128],
                                 Qm[fb][:], start=(fb == 0), stop=(fb == 1))
                    V.tensor_copy(wq[cb][:], wqp[:])

                y_ps = pyp.tile([8, 258], F32, tag="y", name="y_ps")
                for g in range(11):
                    ns = min(4, NL - 4 * g)
                    sT = pm.tile([128, 32], F32, tag="ps", name="sT")
                    for s in range(ns):
                        ci = 4 * g + s
                        o, w = LCH[ci]
                        slot = tokL[ci % NTOKL]
                        tpp = pm.tile([128, 256], F32, tag="ps", name="tpp")
                        for dt in range(2):
                            T.transpose(tpp[0:w, 128 * dt:128 * dt + 128],
                                        tokT[dt][:, o:o + w].bitcast(F32),
                                        id128[:, :].bitcast(F32))
                        S.copy(slot[0:w, 0:256], tpp[0:w, 0:256])
                        bn6 = vp.tile([128, 6], F32, tag="bn6", name="bn6")
                        V.bn_stats(bn6[0:w, :], tpp[0:w, 0:256])
                        V.bn_aggr(stat2[0:w, 2 * ci:2 * ci + 2], bn6[0:w, :])
                        for cb in range(2):
                            T.matmul(sT[0:w, 8 * s:8 * s + 8],
                                     tokT[cb][:, o:o + w], wq[cb][:],
                                     start=(cb == 0), stop=(cb == 1))
                    wg = 128 if ns == 4 else LCH[4 * g][1]
                    S.activation(pT[0:wg, 32 * g:32 * g + 8 * ns],
                                 sT[0:wg, 0:8 * ns], AF.Exp, scale=ISQ)
                    for s in range(ns):
                        ci = 4 * g + s
                        T.matmul(y_ps[:, 0:258], pT[:, 8 * ci:8 * ci + 8],
                                 tokL[ci % NTOKL][:, 0:258],
                                 start=(ci == 0), stop=(ci == NL - 1))

                st2v = stat2[:].rearrange("p (ci two) -> p two ci", two=2)
                S.activation(st2v[:, 1, :], st2v[:, 1, :], AF.Ln, bias=epsc[:, 0:1])
                S.activation(st2v[:, 1, :], st2v[:, 1, :], AF.Exp, scale=-0.5)
                for g in range(11):
                    ns = min(4, NL - 4 * g)
                    mt = pm.tile([2, 512], F32, tag="ps", name="mt")
                    for s in range(ns):
                        ci = 4 * g + s
                        T.transpose(mt[0:2, 128 * s:128 * s + 128],
                                    stat2[:, 2 * ci:2 * ci + 2],
                                    id128[:, :].bitcast(F32))
                    V.tensor_copy(murT[0:2, 512 * g:512 * g + 128 * ns],
                                  mt[0:2, 0:128 * ns])

                srec = vp.tile([8, 1], F32, tag="srec", name="srec")
                V.reciprocal(srec[:], y_ps[:, 256:257])
                ysc = vp.tile([8, 256], F32R, tag="ysc", name="ysc")
                V.tensor_scalar(ysc[:], y_ps[:, 0:256], srec[:], None, ALU.mult)
                yT = [vp.tile([128, 8], F32R, tag=f"yT{d}", name=f"yT{d}") for d in range(2)]
                for cb in range(2):
                    ytp = pm.tile([128, 8], F32R, tag="ps", name="ytp")
                    T.transpose(ytp[0:128, 0:8], ysc[:, 128 * cb:128 * cb + 128],
                                identsb[:, :])
                    V.tensor_copy(yT[cb][:], ytp[:])
                OF = pyp.tile([8, 256], F32, tag="y", name="OF")
                for cb in range(2):
                    T.matmul(OF[:, :], yT[cb][:],
                             qkvo[:, 1024 * cb + 512:1024 * cb + 768],
                             start=(cb == 0), stop=(cb == 1))
                OFs = vp.tile([8, 256], F32R, tag="OFs", name="OFs")
                V.tensor_copy(OFs[:], OF[:, :])

                afl = [vp.tile([128, 2], F32R, tag=f"afl{d}", name=f"afl{d}") for d in range(2)]
                for d in range(2):
                    tpa = pm.tile([128, 8], F32R, tag="ps", name="tpa")
                    T.transpose(tpa[0:128, 0:8], OFs[:, 128 * d:128 * d + 128],
                                identsb[0:8, 0:8])
                    for hh in range(4):
                        r0 = 32 * hh
                        col = 4 * d + hh
                        V.tensor_scalar(
                            afl[d][r0:r0 + 32, 0:2],
                            tpa[r0:r0 + 32, col:col + 1].to_broadcast([32, 2]),
                            tattnb[r0:r0 + 32, 4 + d:4 + d + 1],
                            None, ALU.add)
                for d in range(2):
                    op_ = pm.tile([128, 2], F32, tag="ps", name="ps")
                    mmv(op_[:], qkvo[:, 768 + 128 * d:768 + 128 * d + 128],
                        afl[0][:], start=True, stop=False)
                    mmv(op_[:], qkvo[:, 1024 + 768 + 128 * d:1024 + 768 + 128 * d + 128],
                        afl[1][:], start=False, stop=True)
                    gcol = i * 12 + 0 * 6 + 4 + d
                    V.scalar_tensor_tensor(cls[d][:], op_[:, 0:1],
                                           modpre[:, gcol:gcol + 1],
                                           cls[d][:], ALU.mult, ALU.add)
                    bog = vp.tile([128, 1], F32, tag="bog", name="bog")
                    V.tensor_tensor(bog[:], tattnb[:, 6 + d:6 + d + 1],
                                    modpre[:, gcol:gcol + 1], ALU.mult)
                    V.tensor_tensor(cls[d][:], cls[d][:], bog[:], ALU.add)

                hc2 = cls_ln(i, 1, "hcm")
                ac = [vp.tile([128, 2], F32R, tag=f"ac{m}", name=f"ac{m}") for m in range(8)]
                for m in range(8):
                    ps = pm.tile([128, 2], F32, tag="ps", name="ps")
                    mmv(ps[:], w1c[:, 128 * m:128 * m + 128], hc2[0][:],
                        start=True, stop=False)
                    mmv(ps[:], w1c[:, 1024 + 128 * m:1024 + 128 * m + 128],
                        hc2[1][:], start=False, stop=True)
                    S.activation(ac[m][:], ps[:], AF.Gelu, bias=tb1c[:, m:m + 1])
                for d in range(2):
                    ps = pm.tile([128, 2], F32, tag="ps", name="ps")
                    for k in range(8):
                        mmv(ps[:], w2c[:, 256 * k + 128 * d:256 * k + 128 * d + 128],
                            ac[k][:], start=(k == 0), stop=(k == 7))
                    gcol = i * 12 + 1 * 6 + 4 + d
                    V.scalar_tensor_tensor(cls[d][:], ps[:, 0:1],
                                           modpre[:, gcol:gcol + 1],
                                           cls[d][:], ALU.mult, ALU.add)
                    bog = vp.tile([128, 1], F32, tag="bog", name="bog")
                    V.tensor_tensor(bog[:], tb2c[:, d:d + 1],
                                    modpre[:, gcol:gcol + 1], ALU.mult)
                    V.tensor_tensor(cls[d][:], cls[d][:], bog[:], ALU.add)

                sc2 = [vp.tile([128, 2], F32R, tag=f"sc2{d}", name=f"sc2{d}") for d in range(2)]
                for d in range(2):
                    cond = vp.tile([128, 2], F32R, tag=f"cond{d}", name=f"cond{d}")
                    V.tensor_tensor(cond[:], temb[d][:],
                                    cls[d][:].to_broadcast([128, 2]), ALU.add)
                    th = vp.tile([128, 2], F32, tag=f"th{d}", name=f"th{d}")
                    S.activation(th[:], cond[:], AF.Tanh, scale=0.5)
                    V.tensor_scalar(th[:], th[:], 1.0, None, ALU.add)
                    V.scalar_tensor_tensor(sc2[d][:], cond[:], 0.5, th[:],
                                           ALU.mult, ALU.mult)
                mvec = vp.tile([128, 6], F32, tag="mvec", name="mvec")
                for m in range(6):
                    ps = pm.tile([128, 2], F32, tag="ps", name="ps")
                    mmv(ps[:], mod2[:, 128 * m:128 * m + 128], sc2[0][:],
                        start=True, stop=False)
                    mmv(ps[:], mod2[:, 768 + 128 * m:768 + 128 * m + 128],
                        sc2[1][:], start=False, stop=True)
                    V.tensor_scalar(mvec[:, m:m + 1], ps[:, 0:1], tmodb2[:, m:m + 1],
                                    None, ALU.add)
                av = vp.tile([128, 2], F32, tag="av", name="av")
                bv = vp.tile([128, 4], F32R, tag="bv", name="bv")
                scr2 = vp.tile([128, 1], F32, tag="scr2", name="scr2")
                for d in range(2):
                    lcol = i * 6 + 2 * 2 + d
                    V.tensor_scalar(scr2[:], mvec[:, d:d + 1], 1.0, None, ALU.add)
                    V.tensor_tensor(av[:, d:d + 1], scr2[:],
                                    lngsb[:, lcol:lcol + 1], ALU.mult)
                    V.tensor_tensor(bv[:, 2 * d:2 * d + 2],
                                    scr2[:].to_broadcast([128, 2]),
                                    lnbsb[:, lcol:lcol + 1].to_broadcast([128, 2]),
                                    ALU.mult)
                    V.tensor_tensor(bv[:, 2 * d:2 * d + 2], bv[:, 2 * d:2 * d + 2],
                                    mvec[:, 2 + d:3 + d].to_broadcast([128, 2]),
                                    ALU.add)

                btot = vp.tile([128, 8], F32, tag="btot", name="btot")
                for m in range(8):
                    ps = pm.tile([128, 2], F32, tag="ps", name="ps")
                    mmv(ps[:], w1[:, 128 * m:128 * m + 128], bv[:, 0:2],
                        start=True, stop=False)
                    mmv(ps[:], w1[:, 1024 + 128 * m:1024 + 128 * m + 128],
                        bv[:, 2:4], start=False, stop=True)
                    V.tensor_scalar(btot[:, m:m + 1], ps[:, 0:1], tb1[:, m:m + 1],
                                    None, ALU.add)
                for d in range(2):
                    V.tensor_scalar(w1[:, 1024 * d:1024 * d + 1024],
                                    w1[:, 1024 * d:1024 * d + 1024],
                                    av[:, d:d + 1], None, ALU.mult)

                pend = None
                for gc, (o, w) in enumerate(CHUNKS + [(None, None)]):
                    if o is not None:
                        nsb = (w + 127) // 128
                        mub = pm.tile([128, 512], F32, tag="ps", name="mub")
                        rb = pm.tile([128, 512], F32, tag="ps", name="rb")
                        for s in range(nsb):
                            mc0 = 512 * gc + 128 * s
                            T.matmul(mub[:, 128 * s:128 * s + min(128, w - 128 * s)],
                                     sel[:, 0:128],
                                     murT[0:2, mc0:mc0 + min(128, w - 128 * s)],
                                     start=True, stop=True)
                            T.matmul(rb[:, 128 * s:128 * s + min(128, w - 128 * s)],
                                     sel[:, 128:256],
                                     murT[0:2, mc0:mc0 + min(128, w - 128 * s)],
                                     start=True, stop=True)
                        xh = []
                        for dt in range(2):
                            x_ = cp.tile([128, 512], F32R, tag=f"big{dt}", name=f"xh{dt}")
                            V.tensor_tensor(x_[:, 0:w], tokT[dt][:, o:o + w],
                                            mub[:, 0:w], ALU.subtract)
                            V.tensor_tensor(x_[:, 0:w], x_[:, 0:w], rb[:, 0:w],
                                            ALU.mult)
                            xh.append(x_)
                        A = cp.tile([128, 8 * 512], BF16, tag="A", name="A")
                        for m in range(8):
                            hp = ph1.tile([128, 512], F32, tag="h1", name="h1")
                            T.matmul(hp[:, 0:w], w1[:, 128 * m:128 * m + 128],
                                     xh[0][:, 0:w], start=True, stop=False)
                            T.matmul(hp[:, 0:w], w1[:, 1024 + 128 * m:1024 + 128 * m + 128],
                                     xh[1][:, 0:w], start=False, stop=True)
                            S.activation(A[:, 512 * m:512 * m + w], hp[:, 0:w],
                                         AF.Gelu, bias=btot[:, m:m + 1])
                    if pend is not None:
                        po, pw, pA = pend
                        h2p = [ph2.tile([128, 512], F32, tag=f"h2{d}",
                                        name=f"h2{d}", bufs=1) for d in range(2)]
                        for k in range(8):
                            for d in range(2):
                                T.matmul(h2p[d][:, 0:pw],
                                         w2[:, 256 * k + 128 * d:256 * k + 128 * d + 128],
                                         pA[:, 512 * k:512 * k + pw],
                                         start=(k == 0), stop=False)
                        for d in range(2):
                            T.matmul(h2p[d][:, 0:pw], b2row[0:1, 128 * d:128 * d + 128],
                                     onesr[0:1, 0:pw], start=False, stop=True)
                            gcol = 4 + d
                            V.scalar_tensor_tensor(tokT[d][:, po:po + pw],
                                                   h2p[d][:, 0:pw],
                                                   mvec[:, gcol:gcol + 1],
                                                   tokT[d][:, po:po + pw],
                                                   ALU.mult, ALU.add)
                    pend = (o, w, A) if o is not None else None

            fing = vp.tile([128, 2], F32, tag="fing", name="fing")
            nc.sync.dma_start(fing[:], col2(d_fing, 2))
            finb = vp.tile([128, 2], F32, tag="finb", name="finb")
            nc.sync.dma_start(finb[:], col2(d_finb, 2))
            outw = vp.tile([128, 4], F32R, tag="outw", name="outw")
            ld_split(outw[:], d_outwT, 2)
            outbs = vp.tile([C, 1], F32, tag="outbs", name="outbs")
            nc.sync.dma_start(outbs[:], d_outb[:, :])
            wpr = vp.tile([128, 4], F32R, tag="wpr", name="wpr")
            vb = vp.tile([128, 4], F32R, tag="vb", name="vb")
            for dt in range(2):
                V.tensor_scalar(wpr[:, 2 * dt:2 * dt + 2],
                                outw[:, 2 * dt:2 * dt + 2],
                                fing[:, dt:dt + 1], None, ALU.mult)
                V.tensor_scalar(vb[:, 2 * dt:2 * dt + 2],
                                outw[:, 2 * dt:2 * dt + 2],
                                finb[:, dt:dt + 1], None, ALU.mult)
            pw = pm.tile([2, 4], F32, tag="ps", name="pw")
            for dt in range(2):
                mmv(pw[0:2, 0:2], wpr[:, 2 * dt:2 * dt + 2], onescol[:],
                    start=(dt == 0), stop=(dt == 1))
            for dt in range(2):
                mmv(pw[0:2, 2:4], vb[:, 2 * dt:2 * dt + 2], onescol[:],
                    start=(dt == 0), stop=(dt == 1))
            nws = vp.tile([2, 2], F32, tag="nws", name="nws")
            V.tensor_scalar(nws[0:2, 0:1], pw[0:2, 0:1], -1.0, None, ALU.mult)
            V.tensor_tensor(nws[0:2, 1:2], pw[0:2, 2:3], outbs[:, 0:1], ALU.add)

            for ci, (o, w) in enumerate(LCH):
                tpp = pm.tile([128, 256], F32, tag="ps", name="tppf")
                for dt in range(2):
                    T.transpose(tpp[0:w, 128 * dt:128 * dt + 128],
                                tokT[dt][:, o:o + w].bitcast(F32),
                                id128[:, :].bitcast(F32))
                bn6 = vp.tile([128, 6], F32, tag="bn6", name="bn6")
                V.bn_stats(bn6[0:w, :], tpp[0:w, 0:256])
                V.bn_aggr(stat2[0:w, 2 * ci:2 * ci + 2], bn6[0:w, :])
            st2v = stat2[:].rearrange("p (ci two) -> p two ci", two=2)
            S.activation(st2v[:, 1, :], st2v[:, 1, :], AF.Ln, bias=epsc[:, 0:1])
            S.activation(st2v[:, 1, :], st2v[:, 1, :], AF.Exp, scale=-0.5)

            for g, (o5, w5) in enumerate(CHUNKS):
                ns = (w5 + 127) // 128
                mt = pm.tile([2, 512], F32, tag="ps", name="mt")
                for s in range(ns):
                    ci = 4 * g + s
                    T.transpose(mt[0:2, 128 * s:128 * s + 128],
                                stat2[:, 2 * ci:2 * ci + 2],
                                id128[:, :].bitcast(F32))
                mts = vp.tile([2, 512], F32R, tag="mts", name="mts")
                V.tensor_copy(mts[0:2, 0:128 * ns], mt[0:2, 0:128 * ns])
                z_ps = pm.tile([2, 512], F32, tag="ps", name="z_ps")
                for dt in range(2):
                    T.matmul(z_ps[0:2, 0:w5], wpr[:, 2 * dt:2 * dt + 2],
                             tokT[dt][:, o5:o5 + w5],
                             start=(dt == 0), stop=(dt == 1))
                mr2 = pm.tile([2, 512], F32, tag="ps", name="mr2")
                T.matmul(mr2[0:2, 0:w5], sel[0:2, 0:2], mts[0:2, 0:w5],
                         start=True, stop=True)
                rr2 = pm.tile([2, 512], F32, tag="ps", name="rr2")
                T.matmul(rr2[0:2, 0:w5], sel[0:2, 128:130],
                         mts[0:2, 0:w5], start=True, stop=True)
                zc = vp.tile([2, 512], F32, tag="zc", name="zc")
                V.tensor_copy(zc[0:2, 0:w5], z_ps[0:2, 0:w5])
                t1 = vp.tile([2, 512], F32, tag="t1", name="t1")
                V.scalar_tensor_tensor(t1[0:2, 0:w5], mr2[0:2, 0:w5],
                                       nws[0:2, 0:1], zc[0:2, 0:w5],
                                       ALU.mult, ALU.add)
                ot = cp.tile([C, 512], F32, tag="osb", name="osb", bufs=1)
                V.tensor_tensor(ot[0:2, 0:w5], t1[0:2, 0:w5],
                                rr2[0:2, 0:w5], ALU.mult)
                V.tensor_scalar(ot[0:2, 0:w5], ot[0:2, 0:w5], nws[0:2, 1:2],
                                None, ALU.add)
                nc.sync.dma_start(d_outT[:, o5:o5 + w5], ot[:, 0:w5])

    split_excess_waits(nc)
    return nc


_NC_CACHE = {}


def _get_nc(depth=DEPTH):
    key = depth
    if key not in _NC_CACHE:
        _NC_CACHE[key] = build_nc(depth)
    return _NC_CACHE[key]


def _freqs_hilo():
    f32 = np.float32
    fr = np.exp(
        -np.log(10000.0) * np.arange(TE // 2, dtype=f32) / (TE // 2)
    ).astype(f32)
    hi = (fr.view(np.uint32) & np.uint32(0xFFFFF000)).view(f32)
    lo = (fr - hi).astype(f32)
    return np.stack([hi, lo], axis=1).astype(f32)


def _shared_inputs(inputs):
    f32 = np.float32
    bf16 = _BF16NP
    mb01 = np.asarray(inputs["adaln_mod_b"], dtype=f32)[:, 0:2, :]
    mb01 = mb01.reshape(DEPTH, 2, 6, 128)
    mb01 = np.ascontiguousarray(
        np.transpose(mb01, (3, 2, 0, 1)).reshape(128, 96))
    sh = {
        "posT": np.ascontiguousarray(inputs["pos"][0].T.astype(f32)),
        "inwT": np.ascontiguousarray(inputs["in_w"].T.astype(f32)),
        "inb": inputs["in_b"].reshape(D, 1).astype(f32),
        "freqs": _freqs_hilo(),
        "tp1T": np.ascontiguousarray(inputs["tp1_w"].T.astype(f32)),
        "tp1b": inputs["tp1_b"].reshape(D, 1).astype(f32),
        "tp2T": np.ascontiguousarray(inputs["tp2_w"].T.astype(f32)),
        "tp2b": inputs["tp2_b"].reshape(D, 1).astype(f32),
        "clsv": inputs["cls_tok"].reshape(D, 1).astype(f32),
        "qkvoT": np.ascontiguousarray(
            np.stack(
                [
                    np.stack(
                        [
                            inputs["attn_in_w"][i][0:D].T,
                            inputs["attn_in_w"][i][D:2 * D],
                            inputs["attn_in_w"][i][2 * D:3 * D].T,
                            inputs["attn_out_w"][i].T,
                        ]
                    )
                    for i in range(DEPTH)
                ]
            ).astype(f32)
        ),
        "attnb": np.ascontiguousarray(
            np.stack(
                [
                    np.stack(
                        [
                            inputs["attn_in_b"][i][0:D],
                            inputs["attn_in_b"][i][D:2 * D],
                            inputs["attn_in_b"][i][2 * D:3 * D],
                            inputs["attn_out_b"][i],
                        ]
                    )
                    for i in range(DEPTH)
                ]
            ).astype(f32).reshape(DEPTH, 4, D, 1)
        ),
        "modT": np.ascontiguousarray(
            np.transpose(inputs["adaln_mod_w"], (0, 1, 3, 2)).astype(f32)
        ),
        "modb": inputs["adaln_mod_b"].astype(f32).reshape(DEPTH, 3, 3 * D, 1),
        "modb01": mb01,
        "mod01T": np.ascontiguousarray(
            np.transpose(np.asarray(inputs["adaln_mod_w"], f32)[:, 0:2],
                         (0, 1, 3, 2)).astype(bf16)
        ),
        "lng": inputs["adaln_ln_g"].astype(f32).reshape(DEPTH, 3, D, 1),
        "lnb": inputs["adaln_ln_b"].astype(f32).reshape(DEPTH, 3, D, 1),
        "w1T": np.ascontiguousarray(
            np.transpose(inputs["mlp_w1"], (0, 1, 3, 2)).astype(f32)
        ),
        "b1": inputs["mlp_b1"].astype(f32).reshape(DEPTH, 2, FF, 1),
        "w2T": np.ascontiguousarray(
            np.transpose(inputs["mlp_w2"], (0, 1, 3, 2)).astype(f32)
        ),
        "w2Th": np.ascontiguousarray(
            np.transpose(np.asarray(inputs["mlp_w2"], f32)[:, 1], (2, 0, 1))
            .reshape(FF, DEPTH * D).astype(bf16)
        ),
        "b2": inputs["mlp_b2"].astype(f32).reshape(DEPTH, 2, D, 1),
        "fing": inputs["fin_g"].reshape(D, 1).astype(f32),
        "finb": inputs["fin_b"].reshape(D, 1).astype(f32),
        "outwT": np.ascontiguousarray(inputs["out_w"].T.astype(f32)),
        "outb": inputs["out_b"].reshape(C, 1).astype(f32),
        "ident": np.eye(8, dtype=f32),
        "ident128": np.eye(128, dtype=f32),
        "onessc": np.full((128, 1), 1.0 / 256, dtype=f32),
        "onesb": np.ones((128, 2), dtype=bf16),
        "onesw": np.ones((128, 512), f32),
        "selw": np.concatenate(
            [np.tile(np.array([[1.0], [0.0]], f32), (1, 128)),
             np.tile(np.array([[0.0], [1.0]], f32), (1, 128))], axis=1),
    }
    return sh


def kernel(**inputs):
    global LAST
    nc = _get_nc()
    sh = _shared_inputs(inputs)
    x_t = np.asarray(inputs["x_t"], dtype=np.float32)
    tv = np.asarray(inputs["t"]).astype(np.int32)
    in_maps = []
    for c in range(NCORES):
        m = dict(sh)
        m["xT"] = np.ascontiguousarray(x_t[c].T)
        m["tval"] = tv[c].reshape(1, 1)
        in_maps.append(m)
    res = run_bass_kernel_spmd(
        nc, in_maps, core_ids=list(range(NCORES)), trace=TRACE
    )
    LAST = res
    out = np.stack(
        [np.ascontiguousarray(res.results[c]["outT"].T) for c in range(NCORES)]
    ).astype(np.float32)
    return out


# revision 13
# speedup vs baseline: 1.2308x; 1.0792x over previous
import sys

for _p in ("/opt/trn_rl_repo", "/opt/pypackages"):
    if _p not in sys.path:
        sys.path.append(_p)

import numpy as np
import ml_dtypes

_BF16NP = ml_dtypes.bfloat16
import concourse.bass as bass
import concourse.tile as tile
from concourse import mybir
from concourse.bass_utils import run_bass_kernel_spmd

AF = mybir.ActivationFunctionType
ALU = mybir.AluOpType
F32R = mybir.dt.float32r
F32 = mybir.dt.float32
BF16 = mybir.dt.bfloat16
I32 = mybir.dt.int32

B, L, C, D, H, DEPTH, FF, TE = 8, 5160, 2, 256, 8, 8, 1024, 256
HD = D // H
NCORES = 8
EPS = 1e-5
PI = float(np.pi)
ISQ = float(1.0 / np.sqrt(HD))

CHUNKS = [(i * 512, 512) for i in range(10)] + [(5120, 40)]
LCH = [(i * 128, 128) for i in range(40)] + [(5120, 40)]
NL = len(LCH)

TRACE = False
LAST = None


def split_excess_waits(nc, limit=1):
    fn = nc.m.functions[0]
    blocks = getattr(fn, "instruction_blocks", None) or getattr(fn, "blocks")
    for bb in blocks:
        insts = bb.instructions
        out = []
        for inst in insts:
            si = inst.sync_info
            waits = list(si.on_wait) if si is not None and si.on_wait else []
            if len(waits) > limit:
                keep = waits[-limit:]
                excess = waits[:-limit]
                for i in range(0, len(excess), limit):
                    nop = mybir.InstNoOp(
                        name=nc.get_next_instruction_name(),
                        sync_info=mybir.SyncInfo(
                            on_wait=excess[i:i + limit], on_update=[]
                        ),
                        bass_nofuse=True,
                        engine=inst.engine,
                    )
                    nc.register_instruction(nop)
                    out.append(nop)
                si.on_wait = keep
            out.append(inst)
        if len(out) != len(insts):
            insts[:] = out
    return nc


def build_nc(depth=DEPTH):
    nc = bass.Bass(target_bir_lowering=False, trn_type="TRN2")
    V = nc.vector
    S = nc.scalar
    G = nc.gpsimd
    T = nc.tensor

    def mmv(out, lhsT, rhs2, start, stop):
        T.matmul(out, lhsT, rhs2, start=start, stop=stop)

    d_xT = nc.dram_tensor("xT", [C, L], F32R, kind="ExternalInput")
    d_t = nc.dram_tensor("tval", [1, 1], I32, kind="ExternalInput")
    d_posT = nc.dram_tensor("posT", [D, L], F32, kind="ExternalInput")
    d_inwT = nc.dram_tensor("inwT", [C, D], F32R, kind="ExternalInput")
    d_inb = nc.dram_tensor("inb", [D, 1], F32, kind="ExternalInput")
    d_freqs = nc.dram_tensor("freqs", [TE // 2, 2], F32, kind="ExternalInput")
    d_tp1T = nc.dram_tensor("tp1T", [TE, D], F32R, kind="ExternalInput")
    d_tp1b = nc.dram_tensor("tp1b", [D, 1], F32, kind="ExternalInput")
    d_tp2T = nc.dram_tensor("tp2T", [D, D], F32R, kind="ExternalInput")
    d_tp2b = nc.dram_tensor("tp2b", [D, 1], F32, kind="ExternalInput")
    d_cls = nc.dram_tensor("clsv", [D, 1], F32R, kind="ExternalInput")
    d_qkvoT = nc.dram_tensor("qkvoT", [DEPTH, 4, D, D], BF16, kind="ExternalInput")
    d_attnb = nc.dram_tensor("attnb", [DEPTH, 4, D, 1], F32, kind="ExternalInput")
    d_modT = nc.dram_tensor("modT", [DEPTH, 3, D, 3 * D], F32R, kind="ExternalInput")
    d_mod01T = nc.dram_tensor("mod01T", [DEPTH, 2, D, 3 * D], BF16, kind="ExternalInput")
    d_modb = nc.dram_tensor("modb", [DEPTH, 3, 3 * D, 1], F32, kind="ExternalInput")
    d_modb01 = nc.dram_tensor("modb01", [128, 96], F32, kind="ExternalInput")
    d_lng = nc.dram_tensor("lng", [DEPTH, 3, D, 1], F32, kind="ExternalInput")
    d_lnb = nc.dram_tensor("lnb", [DEPTH, 3, D, 1], F32, kind="ExternalInput")
    d_w1T = nc.dram_tensor("w1T", [DEPTH, 2, D, FF], BF16, kind="ExternalInput")
    d_b1 = nc.dram_tensor("b1", [DEPTH, 2, FF, 1], F32, kind="ExternalInput")
    d_w2T = nc.dram_tensor("w2T", [DEPTH, 2, FF, D], BF16, kind="ExternalInput")
    d_b2 = nc.dram_tensor("b2", [DEPTH, 2, D, 1], F32, kind="ExternalInput")
    d_fing = nc.dram_tensor("fing", [D, 1], F32, kind="ExternalInput")
    d_finb = nc.dram_tensor("finb", [D, 1], F32, kind="ExternalInput")
    d_outwT = nc.dram_tensor("outwT", [D, C], F32R, kind="ExternalInput")
    d_outb = nc.dram_tensor("outb", [C, 1], F32, kind="ExternalInput")
    d_ident = nc.dram_tensor("ident", [8, 8], F32R, kind="ExternalInput")
    d_ident128 = nc.dram_tensor("ident128", [128, 128], F32R, kind="ExternalInput")
    d_sel = nc.dram_tensor("selw", [2, 256], F32R, kind="ExternalInput")
    d_ones = nc.dram_tensor("onesw", [128, 512], F32R, kind="ExternalInput")
    d_onessc = nc.dram_tensor("onessc", [128, 1], F32R, kind="ExternalInput")
    d_onesb = nc.dram_tensor("onesb", [128, 512], BF16, kind="ExternalInput")
    d_b2rh = nc.dram_tensor("b2rh", [DEPTH, D], BF16, kind="ExternalInput")
    d_identb = nc.dram_tensor("identb", [8, 8], BF16, kind="ExternalInput")
    d_outT = nc.dram_tensor("outT", [C, L], F32, kind="ExternalOutput")

    def col2(dram_ap, groups):
        return dram_ap[:, 0].rearrange("(g p) -> p g", p=128)

    def ld_split(dst, dram2d, g):
        x = dram2d.shape[1]
        nc.sync.dma_start(
            dst.rearrange("p (g x) -> p g x", g=g),
            dram2d.rearrange("(g p) x -> p g x", p=128))

    with tile.TileContext(nc) as tc:
        with tc.tile_pool(name="state", bufs=1) as st, \
             tc.tile_pool(name="wts", bufs=2) as wp, \
             tc.tile_pool(name="vecs", bufs=2) as vp, \
             tc.tile_pool(name="chk", bufs=2) as cp, \
             tc.tile_pool(name="ph1", bufs=2, space="PSUM") as ph1, \
             tc.tile_pool(name="ph2", bufs=2, space="PSUM") as ph2, \
             tc.tile_pool(name="py", bufs=1, space="PSUM") as pyp, \
             tc.tile_pool(name="pmisc", bufs=3, space="PSUM") as pm:

            tokT = [st.tile([128, L], F32R, tag=f"tok{d}", name=f"tok{d}") for d in range(2)]
            pT = st.tile([128, NL * 8], BF16, tag="pT", name="pT")
            stat2 = st.tile([128, 2 * NL], F32, tag="stat2", name="stat2")
            murT = st.tile([2, NL * 128], F32R, tag="murT", name="murT")
            sel = st.tile([2, 256], F32R, tag="sel", name="sel")
            onesr = st.tile([1, 512], F32R, tag="onesr", name="onesr")
            onescol = st.tile([128, 2], F32R, tag="onescol", name="onescol")
            cls = [st.tile([128, 1], F32R, tag=f"cls{d}", name=f"cls{d}") for d in range(2)]
            temb = [st.tile([128, 2], F32R, tag=f"temb{d}", name=f"temb{d}") for d in range(2)]
            stm = [st.tile([128, 2], F32R, tag=f"stm{d}", name=f"stm{d}") for d in range(2)]
            modpre = st.tile([128, DEPTH * 12], F32, tag="modpre", name="modpre")
            abpre = st.tile([128, DEPTH * 8], F32, tag="abpre", name="abpre")
            identsb = st.tile([8, 8], F32R, tag="ident", name="ident")
            identb = st.tile([8, 8], BF16, tag="identb", name="identb")
            onesrb = st.tile([1, 512], BF16, tag="onesrb", name="onesrb")
            id128 = st.tile([128, 128], F32R, tag="id128", name="id128")
            onessc = st.tile([128, 1], F32R, tag="onessc", name="onessc")
            epsc = st.tile([128, 1], F32, tag="epsc", name="epsc")
            lngsb = st.tile([128, DEPTH * 6], F32, tag="lngsb", name="lngsb")
            lnbsb = st.tile([128, DEPTH * 6], F32, tag="lnbsb", name="lnbsb")
            NTOKL = 4
            tokL = [st.tile([128, 264], BF16, tag=f"tokL{j}", name=f"tokL{j}")
                    for j in range(NTOKL)]

            nc.sync.dma_start(identsb[:], d_ident[:, :])
            nc.sync.dma_start(identb[:], d_identb[:, :])
            nc.sync.dma_start(onesrb[:], d_onesb[0:1, :])
            nc.sync.dma_start(id128[:], d_ident128[:, :])
            nc.sync.dma_start(onessc[:], d_onessc[:, :])
            nc.sync.dma_start(onesr[:], d_ones[0:1, :])
            nc.sync.dma_start(onescol[:], d_ones[:, 0:2])
            V.memset(pT[:], 0.0)
            V.memset(epsc[:], EPS)
            V.memset(stat2[:], 1.0)
            nc.sync.dma_start(sel[:], d_sel[:, :])
            for j in range(NTOKL):
                nc.sync.dma_start(tokL[j][:, 256:258], d_onesb[:, 0:2])
            for dt in range(2):
                nc.sync.dma_start(
                    lngsb[:].rearrange("p (i g dt) -> p i g dt",
                                       i=DEPTH, g=3)[:, :, :, dt],
                    d_lng[:, :, 128 * dt:128 * dt + 128, 0].rearrange(
                        "i g p -> p i g"))
                nc.sync.dma_start(
                    lnbsb[:].rearrange("p (i g dt) -> p i g dt",
                                       i=DEPTH, g=3)[:, :, :, dt],
                    d_lnb[:, :, 128 * dt:128 * dt + 128, 0].rearrange(
                        "i g p -> p i g"))
            nc.sync.dma_start(cls[0][:], d_cls[0:128, :])
            nc.sync.dma_start(cls[1][:], d_cls[128:256, :])

            with tc.tile_pool(name="pre", bufs=1) as pre:
                tfl = pre.tile([1, 2], F32R, tag="tfl", name="tfl")
                traw = pre.tile([1, 1], I32, tag="traw", name="traw")
                nc.sync.dma_start(traw[:], d_t[:, :])
                V.tensor_copy(tfl[:], traw[:].to_broadcast([1, 2]))
                tb = pm.tile([128, 2], F32, tag="ps", name="tb")
                mmv(tb[:], onesr[0:1, 0:128], tfl[:], start=True, stop=True)
                fsb = pre.tile([128, 2], F32, tag="fsb", name="fsb")
                nc.sync.dma_start(fsb[:], d_freqs[:, :])
                ang = pre.tile([128, 1], F32, tag="ang", name="ang")
                ang2 = pre.tile([128, 1], F32, tag="ang2", name="ang2")
                V.tensor_tensor(ang[:], tb[:, 0:1], fsb[:, 0:1], ALU.mult)
                V.tensor_tensor(ang2[:], tb[:, 0:1], fsb[:, 1:2], ALU.mult)
                V.tensor_tensor(ang[:], ang[:], ang2[:], ALU.add)
                dsc = pre.tile([128, 1], F32, tag="dsc", name="dsc")
                qi = pre.tile([128, 1], I32, tag="qi", name="qi")
                qf = pre.tile([128, 1], F32, tag="qf", name="qf")
                msk = pre.tile([128, 1], F32, tag="msk", name="msk")
                TWO_PI = 2 * PI

                def mod2pi(dst, shift):
                    V.tensor_scalar(dst[:], ang[:], shift, None, ALU.add)
                    V.tensor_scalar(dsc[:], dst[:], 1.0 / TWO_PI, 0.5,
                                    ALU.mult, ALU.subtract)
                    V.tensor_copy(qi[:], dsc[:])
                    V.tensor_copy(qf[:], qi[:])
                    V.scalar_tensor_tensor(dst[:], qf[:], -TWO_PI, dst[:],
                                           ALU.mult, ALU.add)
                    V.tensor_scalar(msk[:], dst[:], TWO_PI, None, ALU.is_ge)
                    V.scalar_tensor_tensor(dst[:], msk[:], -TWO_PI, dst[:],
                                           ALU.mult, ALU.add)
                    V.tensor_scalar(msk[:], dst[:], 0.0, None, ALU.is_lt)
                    V.scalar_tensor_tensor(dst[:], msk[:], TWO_PI, dst[:],
                                           ALU.mult, ALU.add)
                    V.tensor_scalar(dst[:], dst[:], PI, None, ALU.subtract)

                m1 = pre.tile([128, 1], F32, tag="m1", name="m1")
                mod2pi(m1, PI)
                m2 = pre.tile([128, 1], F32, tag="m2", name="m2")
                mod2pi(m2, 1.5 * PI)
                sinf = pre.tile([128, 2], F32R, tag="sinf", name="sinf")
                cosf = pre.tile([128, 2], F32R, tag="cosf", name="cosf")
                S.activation(sinf[:], m1[:].to_broadcast([128, 2]), AF.Sin)
                S.activation(cosf[:], m2[:].to_broadcast([128, 2]), AF.Sin)

                ttp1 = pre.tile([128, 512], F32R, tag="ttp1", name="ttp1")
                ld_split(ttp1[:], d_tp1T, 2)
                ttp2 = pre.tile([128, 512], F32R, tag="ttp2", name="ttp2")
                ld_split(ttp2[:], d_tp2T, 2)
                tp1b = pre.tile([128, 2], F32, tag="tp1b", name="tp1b")
                nc.sync.dma_start(tp1b[:], col2(d_tp1b, 2))
                tp2b = pre.tile([128, 2], F32, tag="tp2b", name="tp2b")
                nc.sync.dma_start(tp2b[:], col2(d_tp2b, 2))

                st1 = [pre.tile([128, 2], F32R, tag=f"st1{m}", name=f"st1{m}") for m in range(2)]
                for m in range(2):
                    ps = pm.tile([128, 2], F32, tag="ps", name="ps")
                    mmv(ps[:], ttp1[:, 128 * m:128 * m + 128], sinf[:],
                        start=True, stop=False)
                    mmv(ps[:], ttp1[:, 256 + 128 * m:256 + 128 * m + 128],
                        cosf[:], start=False, stop=True)
                    S.activation(st1[m][:], ps[:], AF.Silu, bias=tp1b[:, m:m + 1])
                for m in range(2):
                    ps = pm.tile([128, 2], F32, tag="ps", name="ps")
                    mmv(ps[:], ttp2[:, 128 * m:128 * m + 128], st1[0][:],
                        start=True, stop=False)
                    mmv(ps[:], ttp2[:, 256 + 128 * m:256 + 128 * m + 128],
                        st1[1][:], start=False, stop=True)
                    S.activation(temb[m][:], ps[:], AF.Identity,
                                 bias=tp2b[:, m:m + 1])
                    S.activation(stm[m][:], temb[m][:], AF.Silu)

            with tc.tile_pool(name="pre2", bufs=1) as pre:
                inwsb = pre.tile([C, D], F32R, tag="inwsb", name="inwsb")
                nc.sync.dma_start(inwsb[:], d_inwT[:, :])
                inbsb = pre.tile([128, 2], F32, tag="inbsb", name="inbsb")
                nc.sync.dma_start(inbsb[:], col2(d_inb, 2))
                for (o, w) in CHUNKS:
                    xtc = pre.tile([C, 512], F32R, tag="xtc", name="xtc")
                    nc.sync.dma_start(xtc[:, 0:w], d_xT[:, o:o + w])
                    for dt in range(2):
                        ppc = pre.tile([128, 512], F32, tag=f"ppc{dt}", name=f"ppc{dt}")
                        nc.sync.dma_start(ppc[:, 0:w],
                                          d_posT[128 * dt:128 * dt + 128, o:o + w])
                        ps = pm.tile([128, 512], F32, tag="ps", name="ps")
                        T.matmul(ps[:, 0:w], inwsb[:, 128 * dt:128 * dt + 128],
                                 xtc[:, 0:w], start=True, stop=True)
                        V.scalar_tensor_tensor(
                            tokT[dt][:, o:o + w], ps[:, 0:w],
                            inbsb[:, dt:dt + 1], ppc[:, 0:w], ALU.add, ALU.add)

            with tc.tile_pool(name="pre3", bufs=2) as pre:
                for i in range(depth):
                    for g in range(2):
                        tmg = pre.tile([128, 1536], F32R, tag="tmg", name="tmg")
                        ld_split(tmg[:], d_modT[i, g], 2)
                        tmb = pre.tile([128, 6], F32, tag="tmb", name="tmb")
                        nc.sync.dma_start(tmb[:], col2(d_modb[i, g], 6))
                        for m in range(6):
                            ps = pm.tile([128, 2], F32, tag="ps", name="ps")
                            mmv(ps[:], tmg[:, 128 * m:128 * m + 128],
                                stm[0][:], start=True, stop=False)
                            mmv(ps[:], tmg[:, 768 + 128 * m:768 + 128 * m + 128],
                                stm[1][:], start=False, stop=True)
                            colm = i * 12 + g * 6 + m
                            V.tensor_scalar(modpre[:, colm:colm + 1], ps[:, 0:1],
                                            tmb[:, m:m + 1], None, ALU.add)
                        scr = pre.tile([128, 1], F32, tag="scr", name="scr", bufs=1)
                        for dt in range(2):
                            scol = i * 12 + g * 6 + dt
                            shcol = i * 12 + g * 6 + 2 + dt
                            lcol = i * 6 + g * 2 + dt
                            acol = i * 8 + g * 4 + dt
                            bcol = i * 8 + g * 4 + 2 + dt
                            V.tensor_scalar(scr[:], modpre[:, scol:scol + 1],
                                            1.0, None, ALU.add)
                            V.tensor_tensor(abpre[:, acol:acol + 1], scr[:],
                                            lngsb[:, lcol:lcol + 1], ALU.mult)
                            V.tensor_tensor(abpre[:, bcol:bcol + 1], scr[:],
                                            lnbsb[:, lcol:lcol + 1], ALU.mult)
                            V.tensor_tensor(abpre[:, bcol:bcol + 1],
                                            abpre[:, bcol:bcol + 1],
                                            modpre[:, shcol:shcol + 1], ALU.add)

            def cls_ln(i, g, out_tag):
                csc = [vp.tile([128, 2], F32R, tag=f"csc{d}", name=f"csc{d}") for d in range(2)]
                for d in range(2):
                    V.tensor_copy(csc[d][:, 0:1], onessc[:, 0:1])
                    V.tensor_scalar(csc[d][:, 1:2], cls[d][:], 1.0 / 256, None,
                                    ALU.mult)
                ps = pm.tile([1, 2], F32, tag="ps", name="ps")
                for d in range(2):
                    mmv(ps[0:1, 0:2], cls[d][:], csc[d][:, 0:2],
                        start=(d == 0), stop=(d == 1))
                mc = vp.tile([1, 8], F32R, tag="mc", name="mc")
                V.tensor_copy(mc[0:1, 0:2], ps[0:1, 0:2])
                V.tensor_tensor(mc[0:1, 2:3], mc[0:1, 0:1], mc[0:1, 0:1],
                                ALU.mult)
                V.scalar_tensor_tensor(mc[0:1, 3:4], mc[0:1, 1:2], EPS,
                                       mc[0:1, 2:3], ALU.add, ALU.subtract)
                S.activation(mc[0:1, 3:4], mc[0:1, 3:4], AF.Ln)
                S.activation(mc[0:1, 3:4], mc[0:1, 3:4], AF.Exp, scale=-0.5)
                V.tensor_copy(mc[0:1, 4:6], mc[0:1, 0:1].to_broadcast([1, 2]))
                V.tensor_copy(mc[0:1, 6:8], mc[0:1, 3:4].to_broadcast([1, 2]))
                mcb = pm.tile([128, 2], F32, tag="ps", name="mcb")
                rcb = pm.tile([128, 2], F32, tag="ps", name="rcb")
                mmv(mcb[:], onesr[0:1, 0:128], mc[0:1, 4:6],
                    start=True, stop=True)
                mmv(rcb[:], onesr[0:1, 0:128], mc[0:1, 6:8],
                    start=True, stop=True)
                hc = [vp.tile([128, 2], BF16, tag=f"{out_tag}{d}", name=f"{out_tag}{d}") for d in range(2)]
                for d in range(2):
                    acol = i * 8 + g * 4 + d
                    bcol = i * 8 + g * 4 + 2 + d
                    V.tensor_tensor(hc[d][:], cls[d][:].to_broadcast([128, 2]),
                                    mcb[:], ALU.subtract)
                    V.tensor_tensor(hc[d][:], hc[d][:], rcb[:], ALU.mult)
                    V.scalar_tensor_tensor(
                        hc[d][:], hc[d][:], abpre[:, acol:acol + 1],
                        abpre[:, bcol:bcol + 1].to_broadcast([128, 2]),
                        ALU.mult, ALU.add)
                return hc

            for i in range(depth):
                qkvo = wp.tile([128, 2048], BF16, tag="qkvo", name="qkvo")
                for dt in range(2):
                    nc.sync.dma_start(
                        qkvo[:, 1024 * dt:1024 * dt + 1024].rearrange(
                            "p (w x) -> p w x", w=4),
                        d_qkvoT[i][:, 128 * dt:128 * dt + 128, :].rearrange(
                            "w p x -> p w x"))
                w1 = wp.tile([128, 2048], BF16, tag="w1", name="w1")
                ld_split(w1[:], d_w1T[i, 1], 2)
                w2 = wp.tile([128, 2048], BF16, tag="w2", name="w2")
                ld_split(w2[:], d_w2T[i, 1], 8)
                w1c = wp.tile([128, 2048], BF16, tag="w1c", name="w1c", bufs=1)
                ld_split(w1c[:], d_w1T[i, 0], 2)
                w2c = wp.tile([128, 2048], BF16, tag="w2c", name="w2c", bufs=1)
                ld_split(w2c[:], d_w2T[i, 0], 8)
                mod2 = wp.tile([128, 1536], F32R, tag="mod2", name="mod2", bufs=1)
                ld_split(mod2[:], d_modT[i, 2], 2)
                tattnb = vp.tile([128, 8], F32, tag="tattnb", name="tattnb")
                nc.sync.dma_start(
                    tattnb[:].rearrange("p (w dt) -> p w dt", w=4),
                    d_attnb[i][:, :, 0].rearrange("w (dt p) -> p w dt", p=128))
                tb1 = vp.tile([128, 8], F32, tag="tb1", name="tb1")
                nc.sync.dma_start(tb1[:], col2(d_b1[i, 1], 8))
                tb1c = vp.tile([128, 8], F32, tag="tb1c", name="tb1c")
                nc.sync.dma_start(tb1c[:], col2(d_b1[i, 0], 8))
                b2row = vp.tile([1, 256], BF16, tag="b2row", name="b2row", bufs=1)
                nc.sync.dma_start(b2row[:], d_b2rh[i:i + 1, :])
                tb2c = vp.tile([128, 2], F32, tag="tb2c", name="tb2c")
                nc.sync.dma_start(tb2c[:], col2(d_b2[i, 0], 2))
                tmodb2 = vp.tile([128, 6], F32, tag="tmodb2", name="tmodb2")
                nc.sync.dma_start(tmodb2[:], col2(d_modb[i, 2], 6))

                hc = cls_ln(i, 0, "hca")
                Qm = [vp.tile([128, 8], BF16, tag=f"qm{d}", name=f"qm{d}") for d in range(2)]
                for d in range(2):
                    qp = pm.tile([128, 2], F32, tag="ps", name="ps")
                    mmv(qp[:], qkvo[:, 128 * d:128 * d + 128],
                        hc[0][:], start=True, stop=False)
                    mmv(qp[:], qkvo[:, 1024 + 128 * d:1024 + 128 * d + 128],
                        hc[1][:], start=False, stop=True)
                    V.memset(Qm[d][:].bitcast(F32), 0.0)
                    for hh in range(4):
                        r0 = 32 * hh
                        col = 4 * d + hh
                        V.tensor_scalar(Qm[d][r0:r0 + 32, col:col + 1],
                                        qp[r0:r0 + 32, 0:1],
                                        tattnb[r0:r0 + 32, 0 + d:d + 1],
                                        None, ALU.add)
                wq = [vp.tile([128, 8], F32R, tag=f"wq{d}", name=f"wq{d}") for d in range(2)]
                for cb in range(2):
                    wqp = pm.tile([128, 8], F32, tag="ps", name="ps")
                    for fb in range(2):
                        T.matmul(wqp[:],
                                 qkvo[:, 1024 * fb + 256 + 128 * cb:
                                      1024 * fb + 256 + 128 * cb + 128],
                                 Qm[fb][:], start=(fb == 0), stop=(fb == 1))
                    V.tensor_copy(wq[cb][:], wqp[:])

                y_ps = pyp.tile([8, 258], F32, tag="y", name="y_ps")
                for g in range(11):
                    ns = min(4, NL - 4 * g)
                    sT = pm.tile([128, 32], F32, tag="ps", name="sT")
                    for s in range(ns):
                        ci = 4 * g + s
                        o, w = LCH[ci]
                        slot = tokL[ci % NTOKL]
                        tpp = pm.tile([128, 256], F32, tag="ps", name="tpp")
                        for dt in range(2):
                            T.transpose(tpp[0:w, 128 * dt:128 * dt + 128],
                                        tokT[dt][:, o:o + w].bitcast(F32),
                                        id128[:, :].bitcast(F32))
                        S.copy(slot[0:w, 0:256], tpp[0:w, 0:256])
                        bn6 = vp.tile([128, 6], F32, tag="bn6", name="bn6")
                        V.bn_stats(bn6[0:w, :], tpp[0:w, 0:256])
                        V.bn_aggr(stat2[0:w, 2 * ci:2 * ci + 2], bn6[0:w, :])
                        for cb in range(2):
                            T.matmul(sT[0:w, 8 * s:8 * s + 8],
                                     tokT[cb][:, o:o + w], wq[cb][:],
                                     start=(cb == 0), stop=(cb == 1))
                    wg = 128 if ns == 4 else LCH[4 * g][1]
                    S.activation(pT[0:wg, 32 * g:32 * g + 8 * ns],
                                 sT[0:wg, 0:8 * ns], AF.Exp, scale=ISQ)
                    for s in range(ns):
                        ci = 4 * g + s
                        T.matmul(y_ps[:, 0:258], pT[:, 8 * ci:8 * ci + 8],
                                 tokL[ci % NTOKL][:, 0:258],
                                 start=(ci == 0), stop=(ci == NL - 1))

                st2v = stat2[:].rearrange("p (ci two) -> p two ci", two=2)
                S.activation(st2v[:, 1, :], st2v[:, 1, :], AF.Ln, bias=epsc[:, 0:1])
                S.activation(st2v[:, 1, :], st2v[:, 1, :], AF.Exp, scale=-0.5)
                for g in range(11):
                    ns = min(4, NL - 4 * g)
                    mt = pm.tile([2, 512], F32, tag="ps", name="mt")
                    for s in range(ns):
                        ci = 4 * g + s
                        T.transpose(mt[0:2, 128 * s:128 * s + 128],
                                    stat2[:, 2 * ci:2 * ci + 2],
                                    id128[:, :].bitcast(F32))
                    V.tensor_copy(murT[0:2, 512 * g:512 * g + 128 * ns],
                                  mt[0:2, 0:128 * ns])

                srec = vp.tile([8, 1], F32, tag="srec", name="srec")
                V.reciprocal(srec[:], y_ps[:, 256:257])
                ysc = vp.tile([8, 256], BF16, tag="ysc", name="ysc")
                V.tensor_scalar(ysc[:], y_ps[:, 0:256], srec[:], None, ALU.mult)
                yT = [vp.tile([128, 8], BF16, tag=f"yT{d}", name=f"yT{d}") for d in range(2)]
                for cb in range(2):
                    ytp = pm.tile([128, 8], BF16, tag="ps", name="ytp")
                    T.transpose(ytp[0:128, 0:8], ysc[:, 128 * cb:128 * cb + 128],
                                identb[:, :])
                    V.tensor_copy(yT[cb][:], ytp[:])
                OF = pyp.tile([8, 256], F32, tag="y", name="OF")
                for cb in range(2):
                    T.matmul(OF[:, :], yT[cb][:],
                             qkvo[:, 1024 * cb + 512:1024 * cb + 768],
                             start=(cb == 0), stop=(cb == 1))
                OFs = vp.tile([8, 256], BF16, tag="OFs", name="OFs")
                V.tensor_copy(OFs[:], OF[:, :])

                afl = [vp.tile([128, 2], BF16, tag=f"afl{d}", name=f"afl{d}") for d in range(2)]
                for d in range(2):
                    tpa = pm.tile([128, 8], BF16, tag="ps", name="tpa")
                    T.transpose(tpa[0:128, 0:8], OFs[:, 128 * d:128 * d + 128],
                                identb[0:8, 0:8])
                    for hh in range(4):
                        r0 = 32 * hh
                        col = 4 * d + hh
                        V.tensor_scalar(
                            afl[d][r0:r0 + 32, 0:2],
                            tpa[r0:r0 + 32, col:col + 1].to_broadcast([32, 2]),
                            tattnb[r0:r0 + 32, 4 + d:4 + d + 1],
                            None, ALU.add)
                for d in range(2):
                    op_ = pm.tile([128, 2], F32, tag="ps", name="ps")
                    mmv(op_[:], qkvo[:, 768 + 128 * d:768 + 128 * d + 128],
                        afl[0][:], start=True, stop=False)
                    mmv(op_[:], qkvo[:, 1024 + 768 + 128 * d:1024 + 768 + 128 * d + 128],
                        afl[1][:], start=False, stop=True)
                    gcol = i * 12 + 0 * 6 + 4 + d
                    V.scalar_tensor_tensor(cls[d][:], op_[:, 0:1],
                                           modpre[:, gcol:gcol + 1],
                                           cls[d][:], ALU.mult, ALU.add)
                    bog = vp.tile([128, 1], F32, tag="bog", name="bog")
                    V.tensor_tensor(bog[:], tattnb[:, 6 + d:6 + d + 1],
                                    modpre[:, gcol:gcol + 1], ALU.mult)
                    V.tensor_tensor(cls[d][:], cls[d][:], bog[:], ALU.add)

                hc2 = cls_ln(i, 1, "hcm")
                ac = [vp.tile([128, 2], BF16, tag=f"ac{m}", name=f"ac{m}") for m in range(8)]
                for m in range(8):
                    ps = pm.tile([128, 2], F32, tag="ps", name="ps")
                    mmv(ps[:], w1c[:, 128 * m:128 * m + 128], hc2[0][:],
                        start=True, stop=False)
                    mmv(ps[:], w1c[:, 1024 + 128 * m:1024 + 128 * m + 128],
                        hc2[1][:], start=False, stop=True)
                    S.activation(ac[m][:], ps[:], AF.Gelu, bias=tb1c[:, m:m + 1])
                for d in range(2):
                    ps = pm.tile([128, 2], F32, tag="ps", name="ps")
                    for k in range(8):
                        mmv(ps[:], w2c[:, 256 * k + 128 * d:256 * k + 128 * d + 128],
                            ac[k][:], start=(k == 0), stop=(k == 7))
                    gcol = i * 12 + 1 * 6 + 4 + d
                    V.scalar_tensor_tensor(cls[d][:], ps[:, 0:1],
                                           modpre[:, gcol:gcol + 1],
                                           cls[d][:], ALU.mult, ALU.add)
                    bog = vp.tile([128, 1], F32, tag="bog", name="bog")
                    V.tensor_tensor(bog[:], tb2c[:, d:d + 1],
                                    modpre[:, gcol:gcol + 1], ALU.mult)
                    V.tensor_tensor(cls[d][:], cls[d][:], bog[:], ALU.add)

                sc2 = [vp.tile([128, 2], F32R, tag=f"sc2{d}", name=f"sc2{d}") for d in range(2)]
                for d in range(2):
                    cond = vp.tile([128, 2], F32R, tag=f"cond{d}", name=f"cond{d}")
                    V.tensor_tensor(cond[:], temb[d][:],
                                    cls[d][:].to_broadcast([128, 2]), ALU.add)
                    th = vp.tile([128, 2], F32, tag=f"th{d}", name=f"th{d}")
                    S.activation(th[:], cond[:], AF.Tanh, scale=0.5)
                    V.tensor_scalar(th[:], th[:], 1.0, None, ALU.add)
                    V.scalar_tensor_tensor(sc2[d][:], cond[:], 0.5, th[:],
                                           ALU.mult, ALU.mult)
                mvec = vp.tile([128, 6], F32, tag="mvec", name="mvec")
                for m in range(6):
                    ps = pm.tile([128, 2], F32, tag="ps", name="ps")
                    mmv(ps[:], mod2[:, 128 * m:128 * m + 128], sc2[0][:],
                        start=True, stop=False)
                    mmv(ps[:], mod2[:, 768 + 128 * m:768 + 128 * m + 128],
                        sc2[1][:], start=False, stop=True)
                    V.tensor_scalar(mvec[:, m:m + 1], ps[:, 0:1], tmodb2[:, m:m + 1],
                                    None, ALU.add)
                av = vp.tile([128, 2], F32, tag="av", name="av")
                bv = vp.tile([128, 4], BF16, tag="bv", name="bv")
                scr2 = vp.tile([128, 1], F32, tag="scr2", name="scr2")
                for d in range(2):
                    lcol = i * 6 + 2 * 2 + d
                    V.tensor_scalar(scr2[:], mvec[:, d:d + 1], 1.0, None, ALU.add)
                    V.tensor_tensor(av[:, d:d + 1], scr2[:],
                                    lngsb[:, lcol:lcol + 1], ALU.mult)
                    V.tensor_tensor(bv[:, 2 * d:2 * d + 2],
                                    scr2[:].to_broadcast([128, 2]),
                                    lnbsb[:, lcol:lcol + 1].to_broadcast([128, 2]),
                                    ALU.mult)
                    V.tensor_tensor(bv[:, 2 * d:2 * d + 2], bv[:, 2 * d:2 * d + 2],
                                    mvec[:, 2 + d:3 + d].to_broadcast([128, 2]),
                                    ALU.add)

                btot = vp.tile([128, 8], F32, tag="btot", name="btot")
                for m in range(8):
                    ps = pm.tile([128, 2], F32, tag="ps", name="ps")
                    mmv(ps[:], w1[:, 128 * m:128 * m + 128], bv[:, 0:2],
                        start=True, stop=False)
                    mmv(ps[:], w1[:, 1024 + 128 * m:1024 + 128 * m + 128],
                        bv[:, 2:4], start=False, stop=True)
                    V.tensor_scalar(btot[:, m:m + 1], ps[:, 0:1], tb1[:, m:m + 1],
                                    None, ALU.add)
                for d in range(2):
                    V.tensor_scalar(w1[:, 1024 * d:1024 * d + 1024],
                                    w1[:, 1024 * d:1024 * d + 1024],
                                    av[:, d:d + 1], None, ALU.mult)

                pend = None
                for gc, (o, w) in enumerate(CHUNKS + [(None, None)]):
                    if o is not None:
                        nsb = (w + 127) // 128
                        mub = pm.tile([128, 512], F32, tag="ps", name="mub")
                        rb = pm.tile([128, 512], F32, tag="ps", name="rb")
                        for s in range(nsb):
                            mc0 = 512 * gc + 128 * s
                            T.matmul(mub[:, 128 * s:128 * s + min(128, w - 128 * s)],
                                     sel[:, 0:128],
                                     murT[0:2, mc0:mc0 + min(128, w - 128 * s)],
                                     start=True, stop=True)
                            T.matmul(rb[:, 128 * s:128 * s + min(128, w - 128 * s)],
                                     sel[:, 128:256],
                                     murT[0:2, mc0:mc0 + min(128, w - 128 * s)],
                                     start=True, stop=True)
                        xh = []
                        for dt in range(2):
                            x_ = cp.tile([128, 512], BF16, tag=f"big{dt}", name=f"xh{dt}")
                            V.tensor_tensor(x_[:, 0:w], tokT[dt][:, o:o + w],
                                            mub[:, 0:w], ALU.subtract)
                            V.tensor_tensor(x_[:, 0:w], x_[:, 0:w], rb[:, 0:w],
                                            ALU.mult)
                            xh.append(x_)
                        A = cp.tile([128, 8 * 512], BF16, tag="A", name="A")
                        for m in range(8):
                            hp = ph1.tile([128, 512], F32, tag="h1", name="h1")
                            T.matmul(hp[:, 0:w], w1[:, 128 * m:128 * m + 128],
                                     xh[0][:, 0:w], start=True, stop=False)
                            T.matmul(hp[:, 0:w], w1[:, 1024 + 128 * m:1024 + 128 * m + 128],
                                     xh[1][:, 0:w], start=False, stop=True)
                            S.activation(A[:, 512 * m:512 * m + w], hp[:, 0:w],
                                         AF.Gelu, bias=btot[:, m:m + 1])
                    if pend is not None:
                        po, pw, pA = pend
                        h2p = [ph2.tile([128, 512], F32, tag=f"h2{d}",
                                        name=f"h2{d}", bufs=1) for d in range(2)]
                        for k in range(8):
                            for d in range(2):
                                T.matmul(h2p[d][:, 0:pw],
                                         w2[:, 256 * k + 128 * d:256 * k + 128 * d + 128],
                                         pA[:, 512 * k:512 * k + pw],
                                         start=(k == 0), stop=False)
                        for d in range(2):
                            T.matmul(h2p[d][:, 0:pw], b2row[0:1, 128 * d:128 * d + 128],
                                     onesrb[0:1, 0:pw], start=False, stop=True)
                            gcol = 4 + d
                            V.scalar_tensor_tensor(tokT[d][:, po:po + pw],
                                                   h2p[d][:, 0:pw],
                                                   mvec[:, gcol:gcol + 1],
                                                   tokT[d][:, po:po + pw],
                                                   ALU.mult, ALU.add)
                    pend = (o, w, A) if o is not None else None

            fing = vp.tile([128, 2], F32, tag="fing", name="fing")
            nc.sync.dma_start(fing[:], col2(d_fing, 2))
            finb = vp.tile([128, 2], F32, tag="finb", name="finb")
            nc.sync.dma_start(finb[:], col2(d_finb, 2))
            outw = vp.tile([128, 4], F32R, tag="outw", name="outw")
            ld_split(outw[:], d_outwT, 2)
            outbs = vp.tile([C, 1], F32, tag="outbs", name="outbs")
            nc.sync.dma_start(outbs[:], d_outb[:, :])
            wpr = vp.tile([128, 4], F32R, tag="wpr", name="wpr")
            vb = vp.tile([128, 4], F32R, tag="vb", name="vb")
            for dt in range(2):
                V.tensor_scalar(wpr[:, 2 * dt:2 * dt + 2],
                                outw[:, 2 * dt:2 * dt + 2],
                                fing[:, dt:dt + 1], None, ALU.mult)
                V.tensor_scalar(vb[:, 2 * dt:2 * dt + 2],
                                outw[:, 2 * dt:2 * dt + 2],
                                finb[:, dt:dt + 1], None, ALU.mult)
            pw = pm.tile([2, 4], F32, tag="ps", name="pw")
            for dt in range(2):
                mmv(pw[0:2, 0:2], wpr[:, 2 * dt:2 * dt + 2], onescol[:],
                    start=(dt == 0), stop=(dt == 1))
            for dt in range(2):
                mmv(pw[0:2, 2:4], vb[:, 2 * dt:2 * dt + 2], onescol[:],
                    start=(dt == 0), stop=(dt == 1))
            nws = vp.tile([2, 2], F32, tag="nws", name="nws")
            V.tensor_scalar(nws[0:2, 0:1], pw[0:2, 0:1], -1.0, None, ALU.mult)
            V.tensor_tensor(nws[0:2, 1:2], pw[0:2, 2:3], outbs[:, 0:1], ALU.add)

            for ci, (o, w) in enumerate(LCH):
                tpp = pm.tile([128, 256], F32, tag="ps", name="tppf")
                for dt in range(2):
                    T.transpose(tpp[0:w, 128 * dt:128 * dt + 128],
                                tokT[dt][:, o:o + w].bitcast(F32),
                                id128[:, :].bitcast(F32))
                bn6 = vp.tile([128, 6], F32, tag="bn6", name="bn6")
                V.bn_stats(bn6[0:w, :], tpp[0:w, 0:256])
                V.bn_aggr(stat2[0:w, 2 * ci:2 * ci + 2], bn6[0:w, :])
            st2v = stat2[:].rearrange("p (ci two) -> p two ci", two=2)
            S.activation(st2v[:, 1, :], st2v[:, 1, :], AF.Ln, bias=epsc[:, 0:1])
            S.activation(st2v[:, 1, :], st2v[:, 1, :], AF.Exp, scale=-0.5)

            for g, (o5, w5) in enumerate(CHUNKS):
                ns = (w5 + 127) // 128
                mt = pm.tile([2, 512], F32, tag="ps", name="mt")
                for s in range(ns):
                    ci = 4 * g + s
                    T.transpose(mt[0:2, 128 * s:128 * s + 128],
                                stat2[:, 2 * ci:2 * ci + 2],
                                id128[:, :].bitcast(F32))
                mts = vp.tile([2, 512], F32R, tag="mts", name="mts")
                V.tensor_copy(mts[0:2, 0:128 * ns], mt[0:2, 0:128 * ns])
                z_ps = pm.tile([2, 512], F32, tag="ps", name="z_ps")
                for dt in range(2):
                    T.matmul(z_ps[0:2, 0:w5], wpr[:, 2 * dt:2 * dt + 2],
                             tokT[dt][:, o5:o5 + w5],
                             start=(dt == 0), stop=(dt == 1))
                mr2 = pm.tile([2, 512], F32, tag="ps", name="mr2")
                T.matmul(mr2[0:2, 0:w5], sel[0:2, 0:2], mts[0:2, 0:w5],
                         start=True, stop=True)
                rr2 = pm.tile([2, 512], F32, tag="ps", name="rr2")
                T.matmul(rr2[0:2, 0:w5], sel[0:2, 128:130],
                         mts[0:2, 0:w5], start=True, stop=True)
                zc = vp.tile([2, 512], F32, tag="zc", name="zc")
                V.tensor_copy(zc[0:2, 0:w5], z_ps[0:2, 0:w5])
                t1 = vp.tile([2, 512], F32, tag="t1", name="t1")
                V.scalar_tensor_tensor(t1[0:2, 0:w5], mr2[0:2, 0:w5],
                                       nws[0:2, 0:1], zc[0:2, 0:w5],
                                       ALU.mult, ALU.add)
                ot = cp.tile([C, 512], F32, tag="osb", name="osb", bufs=1)
                V.tensor_tensor(ot[0:2, 0:w5], t1[0:2, 0:w5],
                                rr2[0:2, 0:w5], ALU.mult)
                V.tensor_scalar(ot[0:2, 0:w5], ot[0:2, 0:w5], nws[0:2, 1:2],
                                None, ALU.add)
                nc.sync.dma_start(d_outT[:, o5:o5 + w5], ot[:, 0:w5])

    split_excess_waits(nc)
    return nc


_NC_CACHE = {}


def _get_nc(depth=DEPTH):
    key = depth
    if key not in _NC_CACHE:
        _NC_CACHE[key] = build_nc(depth)
    return _NC_CACHE[key]


def _freqs_hilo():
    f32 = np.float32
    fr = np.exp(
        -np.log(10000.0) * np.arange(TE // 2, dtype=f32) / (TE // 2)
    ).astype(f32)
    hi = (fr.view(np.uint32) & np.uint32(0xFFFFF000)).view(f32)
    lo = (fr - hi).astype(f32)
    return np.stack([hi, lo], axis=1).astype(f32)


def _shared_inputs(inputs):
    f32 = np.float32
    bf16 = _BF16NP
    mb01 = np.asarray(inputs["adaln_mod_b"], dtype=f32)[:, 0:2, :]
    mb01 = mb01.reshape(DEPTH, 2, 6, 128)
    mb01 = np.ascontiguousarray(
        np.transpose(mb01, (3, 2, 0, 1)).reshape(128, 96))
    sh = {
        "posT": np.ascontiguousarray(inputs["pos"][0].T.astype(f32)),
        "inwT": np.ascontiguousarray(inputs["in_w"].T.astype(f32)),
        "inb": inputs["in_b"].reshape(D, 1).astype(f32),
        "freqs": _freqs_hilo(),
        "tp1T": np.ascontiguousarray(inputs["tp1_w"].T.astype(f32)),
        "tp1b": inputs["tp1_b"].reshape(D, 1).astype(f32),
        "tp2T": np.ascontiguousarray(inputs["tp2_w"].T.astype(f32)),
        "tp2b": inputs["tp2_b"].reshape(D, 1).astype(f32),
        "clsv": inputs["cls_tok"].reshape(D, 1).astype(f32),
        "qkvoT": np.ascontiguousarray(
            np.stack(
                [
                    np.stack(
                        [
                            inputs["attn_in_w"][i][0:D].T,
                            inputs["attn_in_w"][i][D:2 * D],
                            inputs["attn_in_w"][i][2 * D:3 * D].T,
                            inputs["attn_out_w"][i].T,
                        ]
                    )
                    for i in range(DEPTH)
                ]
            ).astype(bf16)
        ),
        "attnb": np.ascontiguousarray(
            np.stack(
                [
                    np.stack(
                        [
                            inputs["attn_in_b"][i][0:D],
                            inputs["attn_in_b"][i][D:2 * D],
                            inputs["attn_in_b"][i][2 * D:3 * D],
                            inputs["attn_out_b"][i],
                        ]
                    )
                    for i in range(DEPTH)
                ]
            ).astype(f32).reshape(DEPTH, 4, D, 1)
        ),
        "modT": np.ascontiguousarray(
            np.transpose(inputs["adaln_mod_w"], (0, 1, 3, 2)).astype(f32)
        ),
        "modb": inputs["adaln_mod_b"].astype(f32).reshape(DEPTH, 3, 3 * D, 1),
        "modb01": mb01,
        "mod01T": np.ascontiguousarray(
            np.transpose(np.asarray(inputs["adaln_mod_w"], f32)[:, 0:2],
                         (0, 1, 3, 2)).astype(bf16)
        ),
        "lng": inputs["adaln_ln_g"].astype(f32).reshape(DEPTH, 3, D, 1),
        "lnb": inputs["adaln_ln_b"].astype(f32).reshape(DEPTH, 3, D, 1),
        "w1T": np.ascontiguousarray(
            np.transpose(inputs["mlp_w1"], (0, 1, 3, 2)).astype(bf16)
        ),
        "b1": inputs["mlp_b1"].astype(f32).reshape(DEPTH, 2, FF, 1),
        "w2T": np.ascontiguousarray(
            np.transpose(inputs["mlp_w2"], (0, 1, 3, 2)).astype(bf16)
        ),
        "b2": inputs["mlp_b2"].astype(f32).reshape(DEPTH, 2, D, 1),
        "fing": inputs["fin_g"].reshape(D, 1).astype(f32),
        "finb": inputs["fin_b"].reshape(D, 1).astype(f32),
        "outwT": np.ascontiguousarray(inputs["out_w"].T.astype(f32)),
        "outb": inputs["out_b"].reshape(C, 1).astype(f32),
        "ident": np.eye(8, dtype=f32),
        "ident128": np.eye(128, dtype=f32),
        "onessc": np.full((128, 1), 1.0 / 256, dtype=f32),
        "onesb": np.ones((128, 512), dtype=bf16),
        "b2rh": np.asarray(inputs["mlp_b2"], f32)[:, 1].astype(bf16),
        "identb": np.eye(8).astype(bf16),
        "onesw": np.ones((128, 512), f32),
        "selw": np.concatenate(
            [np.tile(np.array([[1.0], [0.0]], f32), (1, 128)),
             np.tile(np.array([[0.0], [1.0]], f32), (1, 128))], axis=1),
    }
    return sh


def kernel(**inputs):
    global LAST
    nc = _get_nc()
    sh = _shared_inputs(inputs)
    x_t = np.asarray(inputs["x_t"], dtype=np.float32)
    tv = np.asarray(inputs["t"]).astype(np.int32)
    in_maps = []
    for c in range(NCORES):
        m = dict(sh)
        m["xT"] = np.ascontiguousarray(x_t[c].T)
        m["tval"] = tv[c].reshape(1, 1)
        in_maps.append(m)
    res = run_bass_kernel_spmd(
        nc, in_maps, core_ids=list(range(NCORES)), trace=TRACE
    )
    LAST = res
    out = np.stack(
        [np.ascontiguousarray(res.results[c]["outT"].T) for c in range(NCORES)]
    ).astype(np.float32)
    return out


# revision 18
# speedup vs baseline: 1.3241x; 1.0757x over previous
import sys

for _p in ("/opt/trn_rl_repo", "/opt/pypackages"):
    if _p not in sys.path:
        sys.path.append(_p)

import numpy as np
import ml_dtypes

_BF16NP = ml_dtypes.bfloat16
import concourse.bass as bass
import concourse.tile as tile
from concourse import mybir
from concourse.bass_utils import run_bass_kernel_spmd

AF = mybir.ActivationFunctionType
ALU = mybir.AluOpType
F32R = mybir.dt.float32r
F32 = mybir.dt.float32
BF16 = mybir.dt.bfloat16
I32 = mybir.dt.int32

B, L, C, D, H, DEPTH, FF, TE = 8, 5160, 2, 256, 8, 8, 1024, 256
HD = D // H
NCORES = 8
EPS = 1e-5
PI = float(np.pi)
ISQ = float(1.0 / np.sqrt(HD))

CHUNKS = [(i * 512, 512) for i in range(10)] + [(5120, 40)]
LCH = [(i * 128, 128) for i in range(40)] + [(5120, 40)]
NL = len(LCH)

TRACE = False
LAST = None


def split_excess_waits(nc, limit=1):
    fn = nc.m.functions[0]
    blocks = getattr(fn, "instruction_blocks", None) or getattr(fn, "blocks")
    for bb in blocks:
        insts = bb.instructions
        out = []
        for inst in insts:
            si = inst.sync_info
            waits = list(si.on_wait) if si is not None and si.on_wait else []
            if len(waits) > limit:
                keep = waits[-limit:]
                excess = waits[:-limit]
                for i in range(0, len(excess), limit):
                    nop = mybir.InstNoOp(
                        name=nc.get_next_instruction_name(),
                        sync_info=mybir.SyncInfo(
                            on_wait=excess[i:i + limit], on_update=[]
                        ),
                        bass_nofuse=True,
                        engine=inst.engine,
                    )
                    nc.register_instruction(nop)
                    out.append(nop)
                si.on_wait = keep
            out.append(inst)
        if len(out) != len(insts):
            insts[:] = out
    return nc


def build_nc(depth=DEPTH):
    nc = bass.Bass(target_bir_lowering=False, trn_type="TRN2")
    V = nc.vector
    S = nc.scalar
    G = nc.gpsimd
    T = nc.tensor

    def mmv(out, lhsT, rhs2, start, stop):
        T.matmul(out, lhsT, rhs2, start=start, stop=stop)

    d_xT = nc.dram_tensor("xT", [C, L], F32R, kind="ExternalInput")
    d_t = nc.dram_tensor("tval", [1, 1], I32, kind="ExternalInput")
    d_posT = nc.dram_tensor("posT", [D, L], F32, kind="ExternalInput")
    d_inwT = nc.dram_tensor("inwT", [C, D], F32R, kind="ExternalInput")
    d_inb = nc.dram_tensor("inb", [D, 1], F32, kind="ExternalInput")
    d_freqs = nc.dram_tensor("freqs", [TE // 2, 2], F32, kind="ExternalInput")
    d_tp1T = nc.dram_tensor("tp1T", [TE, D], F32R, kind="ExternalInput")
    d_tp1b = nc.dram_tensor("tp1b", [D, 1], F32, kind="ExternalInput")
    d_tp2T = nc.dram_tensor("tp2T", [D, D], F32R, kind="ExternalInput")
    d_tp2b = nc.dram_tensor("tp2b", [D, 1], F32, kind="ExternalInput")
    d_cls = nc.dram_tensor("clsv", [D, 1], F32R, kind="ExternalInput")
    d_qkvoT = nc.dram_tensor("qkvoT", [DEPTH, 4, D, D], BF16, kind="ExternalInput")
    d_attnb = nc.dram_tensor("attnb", [DEPTH, 4, D, 1], F32, kind="ExternalInput")
    d_modT = nc.dram_tensor("modT", [DEPTH, 3, D, 3 * D], F32R, kind="ExternalInput")
    d_mod01T = nc.dram_tensor("mod01T", [DEPTH, 2, D, 3 * D], BF16, kind="ExternalInput")
    d_modb = nc.dram_tensor("modb", [DEPTH, 3, 3 * D, 1], F32, kind="ExternalInput")
    d_modb01 = nc.dram_tensor("modb01", [128, 96], F32, kind="ExternalInput")
    d_lng = nc.dram_tensor("lng", [DEPTH, 3, D, 1], F32, kind="ExternalInput")
    d_lnb = nc.dram_tensor("lnb", [DEPTH, 3, D, 1], F32, kind="ExternalInput")
    d_w1T = nc.dram_tensor("w1T", [DEPTH, 2, D, FF], BF16, kind="ExternalInput")
    d_b1 = nc.dram_tensor("b1", [DEPTH, 2, FF, 1], F32, kind="ExternalInput")
    d_w2T = nc.dram_tensor("w2T", [DEPTH, 2, FF, D], BF16, kind="ExternalInput")
    d_b2 = nc.dram_tensor("b2", [DEPTH, 2, D, 1], F32, kind="ExternalInput")
    d_fing = nc.dram_tensor("fing", [D, 1], F32, kind="ExternalInput")
    d_finb = nc.dram_tensor("finb", [D, 1], F32, kind="ExternalInput")
    d_outwT = nc.dram_tensor("outwT", [D, C], F32R, kind="ExternalInput")
    d_outb = nc.dram_tensor("outb", [C, 1], F32, kind="ExternalInput")
    d_ident = nc.dram_tensor("ident", [8, 8], F32R, kind="ExternalInput")
    d_ident128 = nc.dram_tensor("ident128", [128, 128], F32R, kind="ExternalInput")
    d_sel = nc.dram_tensor("selw", [2, 256], F32R, kind="ExternalInput")
    d_sel8 = nc.dram_tensor("sel8w", [8, 1024], F32R, kind="ExternalInput")
    d_ones = nc.dram_tensor("onesw", [128, 512], F32R, kind="ExternalInput")
    d_onessc = nc.dram_tensor("onessc", [128, 1], F32R, kind="ExternalInput")
    d_onesb = nc.dram_tensor("onesb", [128, 512], BF16, kind="ExternalInput")
    d_b2rh = nc.dram_tensor("b2rh", [DEPTH, D], BF16, kind="ExternalInput")
    d_identb = nc.dram_tensor("identb", [8, 8], BF16, kind="ExternalInput")
    d_outT = nc.dram_tensor("outT", [C, L], F32, kind="ExternalOutput")

    def col2(dram_ap, groups):
        return dram_ap[:, 0].rearrange("(g p) -> p g", p=128)

    def ld_split(dst, dram2d, g):
        x = dram2d.shape[1]
        nc.sync.dma_start(
            dst.rearrange("p (g x) -> p g x", g=g),
            dram2d.rearrange("(g p) x -> p g x", p=128))

    with tile.TileContext(nc) as tc:
        with tc.tile_pool(name="state", bufs=1) as st, \
             tc.tile_pool(name="wts", bufs=2) as wp, \
             tc.tile_pool(name="vecs", bufs=2) as vp, \
             tc.tile_pool(name="chk", bufs=2) as cp, \
             tc.tile_pool(name="ph1", bufs=2, space="PSUM") as ph1, \
             tc.tile_pool(name="ph2", bufs=2, space="PSUM") as ph2, \
             tc.tile_pool(name="py", bufs=1, space="PSUM") as pyp, \
             tc.tile_pool(name="pmisc", bufs=3, space="PSUM") as pm:

            tokT = [st.tile([128, L], F32R, tag=f"tok{d}", name=f"tok{d}") for d in range(2)]
            pT = st.tile([128, NL * 8], BF16, tag="pT", name="pT")
            stat2 = st.tile([128, 2 * NL], F32, tag="stat2", name="stat2")
            murT8 = st.tile([8, 11 * 128], F32R, tag="murT8", name="murT8")
            sel8 = st.tile([8, 1024], F32R, tag="sel8", name="sel8")
            sel = st.tile([2, 256], F32R, tag="sel", name="sel")
            onesr = st.tile([1, 512], F32R, tag="onesr", name="onesr")
            onescol = st.tile([128, 2], F32R, tag="onescol", name="onescol")
            cls = [st.tile([128, 1], F32R, tag=f"cls{d}", name=f"cls{d}") for d in range(2)]
            temb = [st.tile([128, 2], F32R, tag=f"temb{d}", name=f"temb{d}") for d in range(2)]
            stm = [st.tile([128, 2], F32R, tag=f"stm{d}", name=f"stm{d}") for d in range(2)]
            modpre = st.tile([128, DEPTH * 12], F32, tag="modpre", name="modpre")
            abpre = st.tile([128, DEPTH * 8], F32, tag="abpre", name="abpre")
            identsb = st.tile([8, 8], F32R, tag="ident", name="ident")
            identb = st.tile([8, 8], BF16, tag="identb", name="identb")
            onesrb = st.tile([1, 512], BF16, tag="onesrb", name="onesrb")
            id128 = st.tile([128, 128], F32R, tag="id128", name="id128")
            onessc = st.tile([128, 1], F32R, tag="onessc", name="onessc")
            epsc = st.tile([128, 1], F32, tag="epsc", name="epsc")
            lngsb = st.tile([128, DEPTH * 6], F32, tag="lngsb", name="lngsb")
            lnbsb = st.tile([128, DEPTH * 6], F32, tag="lnbsb", name="lnbsb")
            NTOKL = 4
            tokL = [st.tile([128, 264], BF16, tag=f"tokL{j}", name=f"tokL{j}")
                    for j in range(NTOKL)]

            nc.sync.dma_start(identsb[:], d_ident[:, :])
            nc.sync.dma_start(identb[:], d_identb[:, :])
            nc.sync.dma_start(onesrb[:], d_onesb[0:1, :])
            nc.sync.dma_start(id128[:], d_ident128[:, :])
            nc.sync.dma_start(onessc[:], d_onessc[:, :])
            nc.sync.dma_start(onesr[:], d_ones[0:1, :])
            nc.sync.dma_start(onescol[:], d_ones[:, 0:2])
            V.memset(pT[:], 0.0)
            V.memset(epsc[:], EPS)
            V.memset(stat2[:], 1.0)
            V.memset(murT8[:].bitcast(F32), 0.0)
            nc.sync.dma_start(sel[:], d_sel[:, :])
            nc.sync.dma_start(sel8[:], d_sel8[:, :])
            for j in range(NTOKL):
                nc.sync.dma_start(tokL[j][:, 256:258], d_onesb[:, 0:2])
            for dt in range(2):
                nc.sync.dma_start(
                    lngsb[:].rearrange("p (i g dt) -> p i g dt",
                                       i=DEPTH, g=3)[:, :, :, dt],
                    d_lng[:, :, 128 * dt:128 * dt + 128, 0].rearrange(
                        "i g p -> p i g"))
                nc.sync.dma_start(
                    lnbsb[:].rearrange("p (i g dt) -> p i g dt",
                                       i=DEPTH, g=3)[:, :, :, dt],
                    d_lnb[:, :, 128 * dt:128 * dt + 128, 0].rearrange(
                        "i g p -> p i g"))
            nc.sync.dma_start(cls[0][:], d_cls[0:128, :])
            nc.sync.dma_start(cls[1][:], d_cls[128:256, :])

            with tc.tile_pool(name="pre", bufs=1) as pre:
                tfl = pre.tile([1, 2], F32R, tag="tfl", name="tfl")
                traw = pre.tile([1, 1], I32, tag="traw", name="traw")
                nc.sync.dma_start(traw[:], d_t[:, :])
                V.tensor_copy(tfl[:], traw[:].to_broadcast([1, 2]))
                tb = pm.tile([128, 2], F32, tag="ps", name="tb")
                mmv(tb[:], onesr[0:1, 0:128], tfl[:], start=True, stop=True)
                fsb = pre.tile([128, 2], F32, tag="fsb", name="fsb")
                nc.sync.dma_start(fsb[:], d_freqs[:, :])
                ang = pre.tile([128, 1], F32, tag="ang", name="ang")
                ang2 = pre.tile([128, 1], F32, tag="ang2", name="ang2")
                V.tensor_tensor(ang[:], tb[:, 0:1], fsb[:, 0:1], ALU.mult)
                V.tensor_tensor(ang2[:], tb[:, 0:1], fsb[:, 1:2], ALU.mult)
                V.tensor_tensor(ang[:], ang[:], ang2[:], ALU.add)
                dsc = pre.tile([128, 1], F32, tag="dsc", name="dsc")
                qi = pre.tile([128, 1], I32, tag="qi", name="qi")
                qf = pre.tile([128, 1], F32, tag="qf", name="qf")
                msk = pre.tile([128, 1], F32, tag="msk", name="msk")
                TWO_PI = 2 * PI

                def mod2pi(dst, shift):
                    V.tensor_scalar(dst[:], ang[:], shift, None, ALU.add)
                    V.tensor_scalar(dsc[:], dst[:], 1.0 / TWO_PI, 0.5,
                                    ALU.mult, ALU.subtract)
                    V.tensor_copy(qi[:], dsc[:])
                    V.tensor_copy(qf[:], qi[:])
                    V.scalar_tensor_tensor(dst[:], qf[:], -TWO_PI, dst[:],
                                           ALU.mult, ALU.add)
                    V.tensor_scalar(msk[:], dst[:], TWO_PI, None, ALU.is_ge)
                    V.scalar_tensor_tensor(dst[:], msk[:], -TWO_PI, dst[:],
                                           ALU.mult, ALU.add)
                    V.tensor_scalar(msk[:], dst[:], 0.0, None, ALU.is_lt)
                    V.scalar_tensor_tensor(dst[:], msk[:], TWO_PI, dst[:],
                                           ALU.mult, ALU.add)
                    V.tensor_scalar(dst[:], dst[:], PI, None, ALU.subtract)

                m1 = pre.tile([128, 1], F32, tag="m1", name="m1")
                mod2pi(m1, PI)
                m2 = pre.tile([128, 1], F32, tag="m2", name="m2")
                mod2pi(m2, 1.5 * PI)
                sinf = pre.tile([128, 2], F32R, tag="sinf", name="sinf")
                cosf = pre.tile([128, 2], F32R, tag="cosf", name="cosf")
                S.activation(sinf[:], m1[:].to_broadcast([128, 2]), AF.Sin)
                S.activation(cosf[:], m2[:].to_broadcast([128, 2]), AF.Sin)

                ttp1 = pre.tile([128, 512], F32R, tag="ttp1", name="ttp1")
                ld_split(ttp1[:], d_tp1T, 2)
                ttp2 = pre.tile([128, 512], F32R, tag="ttp2", name="ttp2")
                ld_split(ttp2[:], d_tp2T, 2)
                tp1b = pre.tile([128, 2], F32, tag="tp1b", name="tp1b")
                nc.sync.dma_start(tp1b[:], col2(d_tp1b, 2))
                tp2b = pre.tile([128, 2], F32, tag="tp2b", name="tp2b")
                nc.sync.dma_start(tp2b[:], col2(d_tp2b, 2))

                st1 = [pre.tile([128, 2], F32R, tag=f"st1{m}", name=f"st1{m}") for m in range(2)]
                for m in range(2):
                    ps = pm.tile([128, 2], F32, tag="ps", name="ps")
                    mmv(ps[:], ttp1[:, 128 * m:128 * m + 128], sinf[:],
                        start=True, stop=False)
                    mmv(ps[:], ttp1[:, 256 + 128 * m:256 + 128 * m + 128],
                        cosf[:], start=False, stop=True)
                    S.activation(st1[m][:], ps[:], AF.Silu, bias=tp1b[:, m:m + 1])
                for m in range(2):
                    ps = pm.tile([128, 2], F32, tag="ps", name="ps")
                    mmv(ps[:], ttp2[:, 128 * m:128 * m + 128], st1[0][:],
                        start=True, stop=False)
                    mmv(ps[:], ttp2[:, 256 + 128 * m:256 + 128 * m + 128],
                        st1[1][:], start=False, stop=True)
                    S.activation(temb[m][:], ps[:], AF.Identity,
                                 bias=tp2b[:, m:m + 1])
                    S.activation(stm[m][:], temb[m][:], AF.Silu)

            with tc.tile_pool(name="pre2", bufs=1) as pre:
                inwsb = pre.tile([C, D], F32R, tag="inwsb", name="inwsb")
                nc.sync.dma_start(inwsb[:], d_inwT[:, :])
                inbsb = pre.tile([128, 2], F32, tag="inbsb", name="inbsb")
                nc.sync.dma_start(inbsb[:], col2(d_inb, 2))
                for (o, w) in CHUNKS:
                    xtc = pre.tile([C, 512], F32R, tag="xtc", name="xtc")
                    nc.sync.dma_start(xtc[:, 0:w], d_xT[:, o:o + w])
                    for dt in range(2):
                        ppc = pre.tile([128, 512], F32, tag=f"ppc{dt}", name=f"ppc{dt}")
                        nc.sync.dma_start(ppc[:, 0:w],
                                          d_posT[128 * dt:128 * dt + 128, o:o + w])
                        ps = pm.tile([128, 512], F32, tag="ps", name="ps")
                        T.matmul(ps[:, 0:w], inwsb[:, 128 * dt:128 * dt + 128],
                                 xtc[:, 0:w], start=True, stop=True)
                        V.scalar_tensor_tensor(
                            tokT[dt][:, o:o + w], ps[:, 0:w],
                            inbsb[:, dt:dt + 1], ppc[:, 0:w], ALU.add, ALU.add)

            with tc.tile_pool(name="pre3", bufs=2) as pre:
                for i in range(depth):
                    for g in range(2):
                        tmg = pre.tile([128, 1536], F32R, tag="tmg", name="tmg")
                        ld_split(tmg[:], d_modT[i, g], 2)
                        tmb = pre.tile([128, 6], F32, tag="tmb", name="tmb")
                        nc.sync.dma_start(tmb[:], col2(d_modb[i, g], 6))
                        for m in range(6):
                            ps = pm.tile([128, 2], F32, tag="ps", name="ps")
                            mmv(ps[:], tmg[:, 128 * m:128 * m + 128],
                                stm[0][:], start=True, stop=False)
                            mmv(ps[:], tmg[:, 768 + 128 * m:768 + 128 * m + 128],
                                stm[1][:], start=False, stop=True)
                            colm = i * 12 + g * 6 + m
                            V.tensor_scalar(modpre[:, colm:colm + 1], ps[:, 0:1],
                                            tmb[:, m:m + 1], None, ALU.add)
                        scr = pre.tile([128, 1], F32, tag="scr", name="scr", bufs=1)
                        for dt in range(2):
                            scol = i * 12 + g * 6 + dt
                            shcol = i * 12 + g * 6 + 2 + dt
                            lcol = i * 6 + g * 2 + dt
                            acol = i * 8 + g * 4 + dt
                            bcol = i * 8 + g * 4 + 2 + dt
                            V.tensor_scalar(scr[:], modpre[:, scol:scol + 1],
                                            1.0, None, ALU.add)
                            V.tensor_tensor(abpre[:, acol:acol + 1], scr[:],
                                            lngsb[:, lcol:lcol + 1], ALU.mult)
                            V.tensor_tensor(abpre[:, bcol:bcol + 1], scr[:],
                                            lnbsb[:, lcol:lcol + 1], ALU.mult)
                            V.tensor_tensor(abpre[:, bcol:bcol + 1],
                                            abpre[:, bcol:bcol + 1],
                                            modpre[:, shcol:shcol + 1], ALU.add)

            def cls_ln(i, g, out_tag):
                csc = [vp.tile([128, 2], F32R, tag=f"csc{d}", name=f"csc{d}") for d in range(2)]
                for d in range(2):
                    V.tensor_copy(csc[d][:, 0:1], onessc[:, 0:1])
                    V.tensor_scalar(csc[d][:, 1:2], cls[d][:], 1.0 / 256, None,
                                    ALU.mult)
                ps = pm.tile([1, 2], F32, tag="ps", name="ps")
                for d in range(2):
                    mmv(ps[0:1, 0:2], cls[d][:], csc[d][:, 0:2],
                        start=(d == 0), stop=(d == 1))
                mc = vp.tile([1, 8], F32R, tag="mc", name="mc")
                V.tensor_copy(mc[0:1, 0:2], ps[0:1, 0:2])
                V.tensor_tensor(mc[0:1, 2:3], mc[0:1, 0:1], mc[0:1, 0:1],
                                ALU.mult)
                V.scalar_tensor_tensor(mc[0:1, 3:4], mc[0:1, 1:2], EPS,
                                       mc[0:1, 2:3], ALU.add, ALU.subtract)
                S.activation(mc[0:1, 3:4], mc[0:1, 3:4], AF.Ln)
                S.activation(mc[0:1, 3:4], mc[0:1, 3:4], AF.Exp, scale=-0.5)
                V.tensor_copy(mc[0:1, 4:6], mc[0:1, 0:1].to_broadcast([1, 2]))
                V.tensor_copy(mc[0:1, 6:8], mc[0:1, 3:4].to_broadcast([1, 2]))
                mcb = pm.tile([128, 2], F32, tag="ps", name="mcb")
                rcb = pm.tile([128, 2], F32, tag="ps", name="rcb")
                mmv(mcb[:], onesr[0:1, 0:128], mc[0:1, 4:6],
                    start=True, stop=True)
                mmv(rcb[:], onesr[0:1, 0:128], mc[0:1, 6:8],
                    start=True, stop=True)
                hc = [vp.tile([128, 2], BF16, tag=f"{out_tag}{d}", name=f"{out_tag}{d}") for d in range(2)]
                for d in range(2):
                    acol = i * 8 + g * 4 + d
                    bcol = i * 8 + g * 4 + 2 + d
                    V.tensor_tensor(hc[d][:], cls[d][:].to_broadcast([128, 2]),
                                    mcb[:], ALU.subtract)
                    V.tensor_tensor(hc[d][:], hc[d][:], rcb[:], ALU.mult)
                    V.scalar_tensor_tensor(
                        hc[d][:], hc[d][:], abpre[:, acol:acol + 1],
                        abpre[:, bcol:bcol + 1].to_broadcast([128, 2]),
                        ALU.mult, ALU.add)
                return hc

            for i in range(depth):
                qkvo = wp.tile([128, 2048], BF16, tag="qkvo", name="qkvo")
                for dt in range(2):
                    nc.sync.dma_start(
                        qkvo[:, 1024 * dt:1024 * dt + 1024].rearrange(
                            "p (w x) -> p w x", w=4),
                        d_qkvoT[i][:, 128 * dt:128 * dt + 128, :].rearrange(
                            "w p x -> p w x"))
                w1 = wp.tile([128, 2048], BF16, tag="w1", name="w1")
                ld_split(w1[:], d_w1T[i, 1], 2)
                w2 = wp.tile([128, 2048], BF16, tag="w2", name="w2")
                ld_split(w2[:], d_w2T[i, 1], 8)
                w1c = wp.tile([128, 2048], BF16, tag="w1c", name="w1c", bufs=1)
                ld_split(w1c[:], d_w1T[i, 0], 2)
                w2c = wp.tile([128, 2048], BF16, tag="w2c", name="w2c", bufs=1)
                ld_split(w2c[:], d_w2T[i, 0], 8)
                mod2 = wp.tile([128, 1536], F32R, tag="mod2", name="mod2", bufs=1)
                ld_split(mod2[:], d_modT[i, 2], 2)
                tattnb = vp.tile([128, 8], F32, tag="tattnb", name="tattnb")
                nc.sync.dma_start(
                    tattnb[:].rearrange("p (w dt) -> p w dt", w=4),
                    d_attnb[i][:, :, 0].rearrange("w (dt p) -> p w dt", p=128))
                tb1 = vp.tile([128, 8], F32, tag="tb1", name="tb1")
                nc.sync.dma_start(tb1[:], col2(d_b1[i, 1], 8))
                tb1c = vp.tile([128, 8], F32, tag="tb1c", name="tb1c")
                nc.sync.dma_start(tb1c[:], col2(d_b1[i, 0], 8))
                tb2t = vp.tile([128, 2], F32, tag="tb2t", name="tb2t")
                nc.sync.dma_start(tb2t[:], col2(d_b2[i, 1], 2))
                tb2c = vp.tile([128, 2], F32, tag="tb2c", name="tb2c")
                nc.sync.dma_start(tb2c[:], col2(d_b2[i, 0], 2))
                tmodb2 = vp.tile([128, 6], F32, tag="tmodb2", name="tmodb2")
                nc.sync.dma_start(tmodb2[:], col2(d_modb[i, 2], 6))

                hc = cls_ln(i, 0, "hca")
                Qm = [vp.tile([128, 8], BF16, tag=f"qm{d}", name=f"qm{d}") for d in range(2)]
                for d in range(2):
                    qp = pm.tile([128, 2], F32, tag="ps", name="ps")
                    mmv(qp[:], qkvo[:, 128 * d:128 * d + 128],
                        hc[0][:], start=True, stop=False)
                    mmv(qp[:], qkvo[:, 1024 + 128 * d:1024 + 128 * d + 128],
                        hc[1][:], start=False, stop=True)
                    V.memset(Qm[d][:].bitcast(F32), 0.0)
                    for hh in range(4):
                        r0 = 32 * hh
                        col = 4 * d + hh
                        V.tensor_scalar(Qm[d][r0:r0 + 32, col:col + 1],
                                        qp[r0:r0 + 32, 0:1],
                                        tattnb[r0:r0 + 32, 0 + d:d + 1],
                                        None, ALU.add)
                wq = [vp.tile([128, 8], F32R, tag=f"wq{d}", name=f"wq{d}") for d in range(2)]
                for cb in range(2):
                    wqp = pm.tile([128, 8], F32, tag="ps", name="ps")
                    for fb in range(2):
                        T.matmul(wqp[:],
                                 qkvo[:, 1024 * fb + 256 + 128 * cb:
                                      1024 * fb + 256 + 128 * cb + 128],
                                 Qm[fb][:], start=(fb == 0), stop=(fb == 1))
                    V.tensor_copy(wq[cb][:], wqp[:])

                y_ps = pyp.tile([8, 258], F32, tag="y", name="y_ps")
                for g in range(11):
                    ns = min(4, NL - 4 * g)
                    sT = pm.tile([128, 32], F32, tag="ps", name="sT")
                    for s in range(ns):
                        ci = 4 * g + s
                        o, w = LCH[ci]
                        slot = tokL[ci % NTOKL]
                        tpp = pm.tile([128, 256], F32, tag="ps", name="tpp")
                        for dt in range(2):
                            T.transpose(tpp[0:w, 128 * dt:128 * dt + 128],
                                        tokT[dt][:, o:o + w].bitcast(F32),
                                        id128[:, :].bitcast(F32))
                        S.copy(slot[0:w, 0:256], tpp[0:w, 0:256])
                        bn6 = vp.tile([128, 6], F32, tag="bn6", name="bn6")
                        V.bn_stats(bn6[0:w, :], tpp[0:w, 0:256])
                        V.bn_aggr(stat2[0:w, 2 * ci:2 * ci + 2], bn6[0:w, :])
                        for cb in range(2):
                            T.matmul(sT[0:w, 8 * s:8 * s + 8],
                                     tokT[cb][:, o:o + w], wq[cb][:],
                                     start=(cb == 0), stop=(cb == 1))
                    wg = 128 if ns == 4 else LCH[4 * g][1]
                    S.activation(pT[0:wg, 32 * g:32 * g + 8 * ns],
                                 sT[0:wg, 0:8 * ns], AF.Exp, scale=ISQ)
                    for s in range(ns):
                        ci = 4 * g + s
                        T.matmul(y_ps[:, 0:258], pT[:, 8 * ci:8 * ci + 8],
                                 tokL[ci % NTOKL][:, 0:258],
                                 start=(ci == 0), stop=(ci == NL - 1))

                st2v = stat2[:].rearrange("p (ci two) -> p two ci", two=2)
                S.activation(st2v[:, 1, :], st2v[:, 1, :], AF.Ln, bias=epsc[:, 0:1])
                S.activation(st2v[:, 1, :], st2v[:, 1, :], AF.Exp, scale=-0.5)
                for g in range(11):
                    ns = min(4, NL - 4 * g)
                    mt = pm.tile([8, 128], F32, tag="ps", name="mt")
                    T.transpose(mt[0:2 * ns, 0:128],
                                stat2[:, 8 * g:8 * g + 2 * ns],
                                id128[:, :].bitcast(F32))
                    V.tensor_copy(murT8[0:2 * ns, 128 * g:128 * g + 128],
                                  mt[0:2 * ns, 0:128])

                srec = vp.tile([8, 1], F32, tag="srec", name="srec")
                V.reciprocal(srec[:], y_ps[:, 256:257])
                ysc = vp.tile([8, 256], BF16, tag="ysc", name="ysc")
                V.tensor_scalar(ysc[:], y_ps[:, 0:256], srec[:], None, ALU.mult)
                yT = [vp.tile([128, 8], BF16, tag=f"yT{d}", name=f"yT{d}") for d in range(2)]
                for cb in range(2):
                    ytp = pm.tile([128, 8], BF16, tag="ps", name="ytp")
                    T.transpose(ytp[0:128, 0:8], ysc[:, 128 * cb:128 * cb + 128],
                                identb[:, :])
                    V.tensor_copy(yT[cb][:], ytp[:])
                OF = pyp.tile([8, 256], F32, tag="y", name="OF")
                for cb in range(2):
                    T.matmul(OF[:, :], yT[cb][:],
                             qkvo[:, 1024 * cb + 512:1024 * cb + 768],
                             start=(cb == 0), stop=(cb == 1))
                OFs = vp.tile([8, 256], BF16, tag="OFs", name="OFs")
                V.tensor_copy(OFs[:], OF[:, :])

                afl = [vp.tile([128, 2], BF16, tag=f"afl{d}", name=f"afl{d}") for d in range(2)]
                for d in range(2):
                    tpa = pm.tile([128, 8], BF16, tag="ps", name="tpa")
                    T.transpose(tpa[0:128, 0:8], OFs[:, 128 * d:128 * d + 128],
                                identb[0:8, 0:8])
                    for hh in range(4):
                        r0 = 32 * hh
                        col = 4 * d + hh
                        V.tensor_scalar(
                            afl[d][r0:r0 + 32, 0:2],
                            tpa[r0:r0 + 32, col:col + 1].to_broadcast([32, 2]),
                            tattnb[r0:r0 + 32, 4 + d:4 + d + 1],
                            None, ALU.add)
                for d in range(2):
                    op_ = pm.tile([128, 2], F32, tag="ps", name="ps")
                    mmv(op_[:], qkvo[:, 768 + 128 * d:768 + 128 * d + 128],
                        afl[0][:], start=True, stop=False)
                    mmv(op_[:], qkvo[:, 1024 + 768 + 128 * d:1024 + 768 + 128 * d + 128],
                        afl[1][:], start=False, stop=True)
                    gcol = i * 12 + 0 * 6 + 4 + d
                    V.scalar_tensor_tensor(cls[d][:], op_[:, 0:1],
                                           modpre[:, gcol:gcol + 1],
                                           cls[d][:], ALU.mult, ALU.add)
                    bog = vp.tile([128, 1], F32, tag="bog", name="bog")
                    V.tensor_tensor(bog[:], tattnb[:, 6 + d:6 + d + 1],
                                    modpre[:, gcol:gcol + 1], ALU.mult)
                    V.tensor_tensor(cls[d][:], cls[d][:], bog[:], ALU.add)

                hc2 = cls_ln(i, 1, "hcm")
                ac = [vp.tile([128, 2], BF16, tag=f"ac{m}", name=f"ac{m}") for m in range(8)]
                for m in range(8):
                    ps = pm.tile([128, 2], F32, tag="ps", name="ps")
                    mmv(ps[:], w1c[:, 128 * m:128 * m + 128], hc2[0][:],
                        start=True, stop=False)
                    mmv(ps[:], w1c[:, 1024 + 128 * m:1024 + 128 * m + 128],
                        hc2[1][:], start=False, stop=True)
                    S.activation(ac[m][:], ps[:], AF.Gelu, bias=tb1c[:, m:m + 1])
                for d in range(2):
                    ps = pm.tile([128, 2], F32, tag="ps", name="ps")
                    for k in range(8):
                        mmv(ps[:], w2c[:, 256 * k + 128 * d:256 * k + 128 * d + 128],
                            ac[k][:], start=(k == 0), stop=(k == 7))
                    gcol = i * 12 + 1 * 6 + 4 + d
                    V.scalar_tensor_tensor(cls[d][:], ps[:, 0:1],
                                           modpre[:, gcol:gcol + 1],
                                           cls[d][:], ALU.mult, ALU.add)
                    bog = vp.tile([128, 1], F32, tag="bog", name="bog")
                    V.tensor_tensor(bog[:], tb2c[:, d:d + 1],
                                    modpre[:, gcol:gcol + 1], ALU.mult)
                    V.tensor_tensor(cls[d][:], cls[d][:], bog[:], ALU.add)

                sc2 = [vp.tile([128, 2], F32R, tag=f"sc2{d}", name=f"sc2{d}") for d in range(2)]
                for d in range(2):
                    cond = vp.tile([128, 2], F32R, tag=f"cond{d}", name=f"cond{d}")
                    V.tensor_tensor(cond[:], temb[d][:],
                                    cls[d][:].to_broadcast([128, 2]), ALU.add)
                    th = vp.tile([128, 2], F32, tag=f"th{d}", name=f"th{d}")
                    S.activation(th[:], cond[:], AF.Tanh, scale=0.5)
                    V.tensor_scalar(th[:], th[:], 1.0, None, ALU.add)
                    V.scalar_tensor_tensor(sc2[d][:], cond[:], 0.5, th[:],
                                           ALU.mult, ALU.mult)
                mvec = vp.tile([128, 6], F32, tag="mvec", name="mvec")
                for m in range(6):
                    ps = pm.tile([128, 2], F32, tag="ps", name="ps")
                    mmv(ps[:], mod2[:, 128 * m:128 * m + 128], sc2[0][:],
                        start=True, stop=False)
                    mmv(ps[:], mod2[:, 768 + 128 * m:768 + 128 * m + 128],
                        sc2[1][:], start=False, stop=True)
                    V.tensor_scalar(mvec[:, m:m + 1], ps[:, 0:1], tmodb2[:, m:m + 1],
                                    None, ALU.add)
                av = vp.tile([128, 2], F32, tag="av", name="av")
                bv = vp.tile([128, 4], BF16, tag="bv", name="bv")
                scr2 = vp.tile([128, 1], F32, tag="scr2", name="scr2")
                for d in range(2):
                    lcol = i * 6 + 2 * 2 + d
                    V.tensor_scalar(scr2[:], mvec[:, d:d + 1], 1.0, None, ALU.add)
                    V.tensor_tensor(av[:, d:d + 1], scr2[:],
                                    lngsb[:, lcol:lcol + 1], ALU.mult)
                    V.tensor_tensor(bv[:, 2 * d:2 * d + 2],
                                    scr2[:].to_broadcast([128, 2]),
                                    lnbsb[:, lcol:lcol + 1].to_broadcast([128, 2]),
                                    ALU.mult)
                    V.tensor_tensor(bv[:, 2 * d:2 * d + 2], bv[:, 2 * d:2 * d + 2],
                                    mvec[:, 2 + d:3 + d].to_broadcast([128, 2]),
                                    ALU.add)

                b2g = vp.tile([128, 2], F32, tag="b2g", name="b2g")
                for d in range(2):
                    V.tensor_tensor(b2g[:, d:d + 1], tb2t[:, d:d + 1],
                                    mvec[:, 4 + d:5 + d], ALU.mult)
                btot = vp.tile([128, 8], F32, tag="btot", name="btot")
                for m in range(8):
                    ps = pm.tile([128, 2], F32, tag="ps", name="ps")
                    mmv(ps[:], w1[:, 128 * m:128 * m + 128], bv[:, 0:2],
                        start=True, stop=False)
                    mmv(ps[:], w1[:, 1024 + 128 * m:1024 + 128 * m + 128],
                        bv[:, 2:4], start=False, stop=True)
                    V.tensor_scalar(btot[:, m:m + 1], ps[:, 0:1], tb1[:, m:m + 1],
                                    None, ALU.add)
                for d in range(2):
                    V.tensor_scalar(w1[:, 1024 * d:1024 * d + 1024],
                                    w1[:, 1024 * d:1024 * d + 1024],
                                    av[:, d:d + 1], None, ALU.mult)

                pend = None
                for gc, (o, w) in enumerate(CHUNKS + [(None, None)]):
                    if o is not None:
                        nsb = (w + 127) // 128
                        mub = pm.tile([128, 512], F32, tag="ps", name="mub")
                        rb = pm.tile([128, 512], F32, tag="ps", name="rb")
                        grp = murT8[0:8, 128 * gc:128 * gc + 128]
                        for s in range(nsb):
                            ws = min(128, w - 128 * s)
                            T.matmul(mub[:, 128 * s:128 * s + ws],
                                     sel8[:, 128 * s:128 * s + 128],
                                     grp[:, 0:ws], start=True, stop=True)
                            T.matmul(rb[:, 128 * s:128 * s + ws],
                                     sel8[:, 512 + 128 * s:512 + 128 * s + 128],
                                     grp[:, 0:ws], start=True, stop=True)
                        xh = []
                        for dt in range(2):
                            x_ = cp.tile([128, 512], BF16, tag=f"big{dt}", name=f"xh{dt}")
                            V.tensor_tensor(x_[:, 0:w], tokT[dt][:, o:o + w],
                                            mub[:, 0:w], ALU.subtract)
                            V.tensor_tensor(x_[:, 0:w], x_[:, 0:w], rb[:, 0:w],
                                            ALU.mult)
                            xh.append(x_)
                        A = cp.tile([128, 8 * 512], BF16, tag="A", name="A")
                        for m in range(8):
                            hp = ph1.tile([128, 512], F32, tag="h1", name="h1")
                            T.matmul(hp[:, 0:w], w1[:, 128 * m:128 * m + 128],
                                     xh[0][:, 0:w], start=True, stop=False)
                            T.matmul(hp[:, 0:w], w1[:, 1024 + 128 * m:1024 + 128 * m + 128],
                                     xh[1][:, 0:w], start=False, stop=True)
                            S.activation(A[:, 512 * m:512 * m + w], hp[:, 0:w],
                                         AF.Gelu, bias=btot[:, m:m + 1])
                    if pend is not None:
                        po, pw, pA = pend
                        h2p = [ph2.tile([128, 512], F32, tag=f"h2{d}",
                                        name=f"h2{d}", bufs=1) for d in range(2)]
                        for k in range(8):
                            for d in range(2):
                                T.matmul(h2p[d][:, 0:pw],
                                         w2[:, 256 * k + 128 * d:256 * k + 128 * d + 128],
                                         pA[:, 512 * k:512 * k + pw],
                                         start=(k == 0), stop=(k == 7))
                        for d in range(2):
                            gcol = 4 + d
                            dl = cp.tile([128, 512], F32, tag=f"dl{d}", name=f"dl{d}")
                            V.scalar_tensor_tensor(
                                dl[:, 0:pw], h2p[d][:, 0:pw],
                                mvec[:, gcol:gcol + 1],
                                b2g[:, d:d + 1].to_broadcast([128, pw]),
                                ALU.mult, ALU.add)
                            V.tensor_tensor(tokT[d][:, po:po + pw],
                                            tokT[d][:, po:po + pw],
                                            dl[:, 0:pw], ALU.add)
                    pend = (o, w, A) if o is not None else None

            fing = vp.tile([128, 2], F32, tag="fing", name="fing")
            nc.sync.dma_start(fing[:], col2(d_fing, 2))
            finb = vp.tile([128, 2], F32, tag="finb", name="finb")
            nc.sync.dma_start(finb[:], col2(d_finb, 2))
            outw = vp.tile([128, 4], F32R, tag="outw", name="outw")
            ld_split(outw[:], d_outwT, 2)
            outbs = vp.tile([C, 1], F32, tag="outbs", name="outbs")
            nc.sync.dma_start(outbs[:], d_outb[:, :])
            wpr = vp.tile([128, 4], F32R, tag="wpr", name="wpr")
            vb = vp.tile([128, 4], F32R, tag="vb", name="vb")
            for dt in range(2):
                V.tensor_scalar(wpr[:, 2 * dt:2 * dt + 2],
                                outw[:, 2 * dt:2 * dt + 2],
                                fing[:, dt:dt + 1], None, ALU.mult)
                V.tensor_scalar(vb[:, 2 * dt:2 * dt + 2],
                                outw[:, 2 * dt:2 * dt + 2],
                                finb[:, dt:dt + 1], None, ALU.mult)
            pw = pm.tile([2, 4], F32, tag="ps", name="pw")
            for dt in range(2):
                mmv(pw[0:2, 0:2], wpr[:, 2 * dt:2 * dt + 2], onescol[:],
                    start=(dt == 0), stop=(dt == 1))
            for dt in range(2):
                mmv(pw[0:2, 2:4], vb[:, 2 * dt:2 * dt + 2], onescol[:],
                    start=(dt == 0), stop=(dt == 1))
            nws = vp.tile([2, 2], F32, tag="nws", name="nws")
            V.tensor_scalar(nws[0:2, 0:1], pw[0:2, 0:1], -1.0, None, ALU.mult)
            V.tensor_tensor(nws[0:2, 1:2], pw[0:2, 2:3], outbs[:, 0:1], ALU.add)

            for ci, (o, w) in enumerate(LCH):
                tpp = pm.tile([128, 256], F32, tag="ps", name="tppf")
                for dt in range(2):
                    T.transpose(tpp[0:w, 128 * dt:128 * dt + 128],
                                tokT[dt][:, o:o + w].bitcast(F32),
                                id128[:, :].bitcast(F32))
                bn6 = vp.tile([128, 6], F32, tag="bn6", name="bn6")
                V.bn_stats(bn6[0:w, :], tpp[0:w, 0:256])
                V.bn_aggr(stat2[0:w, 2 * ci:2 * ci + 2], bn6[0:w, :])
            st2v = stat2[:].rearrange("p (ci two) -> p two ci", two=2)
            S.activation(st2v[:, 1, :], st2v[:, 1, :], AF.Ln, bias=epsc[:, 0:1])
            S.activation(st2v[:, 1, :], st2v[:, 1, :], AF.Exp, scale=-0.5)

            for g, (o5, w5) in enumerate(CHUNKS):
                ns = (w5 + 127) // 128
                mt = pm.tile([2, 512], F32, tag="ps", name="mt")
                for s in range(ns):
                    ci = 4 * g + s
                    T.transpose(mt[0:2, 128 * s:128 * s + 128],
                                stat2[:, 2 * ci:2 * ci + 2],
                                id128[:, :].bitcast(F32))
                mts = vp.tile([2, 512], F32R, tag="mts", name="mts")
                V.tensor_copy(mts[0:2, 0:128 * ns], mt[0:2, 0:128 * ns])
                z_ps = pm.tile([2, 512], F32, tag="ps", name="z_ps")
                for dt in range(2):
                    T.matmul(z_ps[0:2, 0:w5], wpr[:, 2 * dt:2 * dt + 2],
                             tokT[dt][:, o5:o5 + w5],
                             start=(dt == 0), stop=(dt == 1))
                mr2 = pm.tile([2, 512], F32, tag="ps", name="mr2")
                T.matmul(mr2[0:2, 0:w5], sel[0:2, 0:2], mts[0:2, 0:w5],
                         start=True, stop=True)
                rr2 = pm.tile([2, 512], F32, tag="ps", name="rr2")
                T.matmul(rr2[0:2, 0:w5], sel[0:2, 128:130],
                         mts[0:2, 0:w5], start=True, stop=True)
                zc = vp.tile([2, 512], F32, tag="zc", name="zc")
                V.tensor_copy(zc[0:2, 0:w5], z_ps[0:2, 0:w5])
                t1 = vp.tile([2, 512], F32, tag="t1", name="t1")
                V.scalar_tensor_tensor(t1[0:2, 0:w5], mr2[0:2, 0:w5],
                                       nws[0:2, 0:1], zc[0:2, 0:w5],
                                       ALU.mult, ALU.add)
                ot = cp.tile([C, 512], F32, tag="osb", name="osb", bufs=1)
                V.tensor_tensor(ot[0:2, 0:w5], t1[0:2, 0:w5],
                                rr2[0:2, 0:w5], ALU.mult)
                V.tensor_scalar(ot[0:2, 0:w5], ot[0:2, 0:w5], nws[0:2, 1:2],
                                None, ALU.add)
                nc.sync.dma_start(d_outT[:, o5:o5 + w5], ot[:, 0:w5])

    split_excess_waits(nc)
    return nc


_NC_CACHE = {}


def _get_nc(depth=DEPTH):
    key = depth
    if key not in _NC_CACHE:
        _NC_CACHE[key] = build_nc(depth)
    return _NC_CACHE[key]


def _freqs_hilo():
    f32 = np.float32
    fr = np.exp(
        -np.log(10000.0) * np.arange(TE // 2, dtype=f32) / (TE // 2)
    ).astype(f32)
    hi = (fr.view(np.uint32) & np.uint32(0xFFFFF000)).view(f32)
    lo = (fr - hi).astype(f32)
    return np.stack([hi, lo], axis=1).astype(f32)



def _sel8():
    w = np.zeros((8, 1024), np.float32)
    for sblk in range(4):
        w[2 * sblk, 128 * sblk:128 * sblk + 128] = 1.0
        w[2 * sblk + 1, 512 + 128 * sblk:512 + 128 * sblk + 128] = 1.0
    return w


def _shared_inputs(inputs):
    f32 = np.float32
    bf16 = _BF16NP
    mb01 = np.asarray(inputs["adaln_mod_b"], dtype=f32)[:, 0:2, :]
    mb01 = mb01.reshape(DEPTH, 2, 6, 128)
    mb01 = np.ascontiguousarray(
        np.transpose(mb01, (3, 2, 0, 1)).reshape(128, 96))
    sh = {
        "posT": np.ascontiguousarray(inputs["pos"][0].T.astype(f32)),
        "inwT": np.ascontiguousarray(inputs["in_w"].T.astype(f32)),
        "inb": inputs["in_b"].reshape(D, 1).astype(f32),
        "freqs": _freqs_hilo(),
        "tp1T": np.ascontiguousarray(inputs["tp1_w"].T.astype(f32)),
        "tp1b": inputs["tp1_b"].reshape(D, 1).astype(f32),
        "tp2T": np.ascontiguousarray(inputs["tp2_w"].T.astype(f32)),
        "tp2b": inputs["tp2_b"].reshape(D, 1).astype(f32),
        "clsv": inputs["cls_tok"].reshape(D, 1).astype(f32),
        "qkvoT": np.ascontiguousarray(
            np.stack(
                [
                    np.stack(
                        [
                            inputs["attn_in_w"][i][0:D].T,
                            inputs["attn_in_w"][i][D:2 * D],
                            inputs["attn_in_w"][i][2 * D:3 * D].T,
                            inputs["attn_out_w"][i].T,
                        ]
                    )
                    for i in range(DEPTH)
                ]
            ).astype(bf16)
        ),
        "attnb": np.ascontiguousarray(
            np.stack(
                [
                    np.stack(
                        [
                            inputs["attn_in_b"][i][0:D],
                            inputs["attn_in_b"][i][D:2 * D],
                            inputs["attn_in_b"][i][2 * D:3 * D],
                            inputs["attn_out_b"][i],
                        ]
                    )
                    for i in range(DEPTH)
                ]
            ).astype(f32).reshape(DEPTH, 4, D, 1)
        ),
        "modT": np.ascontiguousarray(
            np.transpose(inputs["adaln_mod_w"], (0, 1, 3, 2)).astype(f32)
        ),
        "modb": inputs["adaln_mod_b"].astype(f32).reshape(DEPTH, 3, 3 * D, 1),
        "modb01": mb01,
        "mod01T": np.ascontiguousarray(
            np.transpose(np.asarray(inputs["adaln_mod_w"], f32)[:, 0:2],
                         (0, 1, 3, 2)).astype(bf16)
        ),
        "lng": inputs["adaln_ln_g"].astype(f32).reshape(DEPTH, 3, D, 1),
        "lnb": inputs["adaln_ln_b"].astype(f32).reshape(DEPTH, 3, D, 1),
        "w1T": np.ascontiguousarray(
            np.transpose(inputs["mlp_w1"], (0, 1, 3, 2)).astype(bf16)
        ),
        "b1": inputs["mlp_b1"].astype(f32).reshape(DEPTH, 2, FF, 1),
        "w2T": np.ascontiguousarray(
            np.transpose(inputs["mlp_w2"], (0, 1, 3, 2)).astype(bf16)
        ),
        "b2": inputs["mlp_b2"].astype(f32).reshape(DEPTH, 2, D, 1),
        "fing": inputs["fin_g"].reshape(D, 1).astype(f32),
        "finb": inputs["fin_b"].reshape(D, 1).astype(f32),
        "outwT": np.ascontiguousarray(inputs["out_w"].T.astype(f32)),
        "outb": inputs["out_b"].reshape(C, 1).astype(f32),
        "ident": np.eye(8, dtype=f32),
        "ident128": np.eye(128, dtype=f32),
        "onessc": np.full((128, 1), 1.0 / 256, dtype=f32),
        "onesb": np.ones((128, 512), dtype=bf16),
        "sel8w": _sel8(),
        "b2rh": np.asarray(inputs["mlp_b2"], f32)[:, 1].astype(bf16),
        "identb": np.eye(8).astype(bf16),
        "onesw": np.ones((128, 512), f32),
        "selw": np.concatenate(
            [np.tile(np.array([[1.0], [0.0]], f32), (1, 128)),
             np.tile(np.array([[0.0], [1.0]], f32), (1, 128))], axis=1),
    }
    return sh


def kernel(**inputs):
    global LAST
    nc = _get_nc()
    sh = _shared_inputs(inputs)
    x_t = np.asarray(inputs["x_t"], dtype=np.float32)
    tv = np.asarray(inputs["t"]).astype(np.int32)
    in_maps = []
    for c in range(NCORES):
        m = dict(sh)
        m["xT"] = np.ascontiguousarray(x_t[c].T)
        m["tval"] = tv[c].reshape(1, 1)
        in_maps.append(m)
    res = run_bass_kernel_spmd(
        nc, in_maps, core_ids=list(range(NCORES)), trace=TRACE
    )
    LAST = res
    out = np.stack(
        [np.ascontiguousarray(res.results[c]["outT"].T) for c in range(NCORES)]
    ).astype(np.float32)
    return out
